# revision 2
# baseline (speedup 1.0000x reference)
"""GCN (4-layer message-passing + linear head) on 8 Trainium2 NeuronCores.

v4 design — non-transpose dma_gather + per-column PE transpose + DVE
segment reduce:

* Node partitioning as v2: nodes ranked by global in-degree, rank%8 = core,
  k_local = rank//8; per-node state feature-major in SBUF.
* Per layer the node-major bf16 table [40960, 128] is AllGathered as in v2.
* Messages are fetched with dma_gather(transpose=False): 256B contiguous
  rows land row i -> (partition i%128, free slot i//128) at DMA-bus speed
  (the v2 XBAR spray-transpose RX path ran ~7 ns/row).
* Edges are sorted by dst and packed into 128-row columns: per column,
  khat consecutive dsts x s slots (s = max edge count over the 8 cores for
  those dsts, khat*s <= 128); pad slots gather zero rows (spread over all
  zero rows to avoid a single-address HBM hotspot).
* Per column, one PE transpose (self-loading matmul against a bf16
  identity) lands the column feature-major in a PSUM tile (8 columns per
  bank, bf16); DVE tensor_reduce then sums each dst's contiguous s-slot
  stripe, writing z feature-major directly: lo-stream reduces write z,
  hi-stream reduces go to a scratch tile and are added.

Math (as v2): deg = indeg+1, dinv = deg^-1/2,
  y = relu(dinv*[segsum(g[src]) + g[v]] + b),  g = (dinv*x) @ W.
"""

import os
import sys
import numpy as np

for _p in ("/opt/trn_rl_repo",):
    if os.path.isdir(_p) and _p not in sys.path:
        sys.path.insert(0, _p)

# ----------------------------------------------------------------------------
# Problem constants
# ----------------------------------------------------------------------------
N = 40000
E = 640000
F_IN = 128
H = 64
C_OUT = 32
M = 8
NPC = 5000
TILES = 40
COLS = TILES * 128          # 5120
SH = COLS
TBL = SH * M                # 40960
VIEW_A = (0, 32768)
VIEW_B = (TBL - 32768, TBL)
CALL_COLS = 24              # max gather-call size in 128-row columns
PTCOLS = 8                  # transposed columns per PSUM tile (bf16)
LAYER_DIMS = [(F_IN, H), (H, 2 * H), (2 * H, 2 * H), (2 * H, H)]


def _row_in_shard(k):
    # row = k directly: high-degree (low-k) rows cluster at the shard front
    # for HBM row-buffer locality; zero rows [NPC, SH) are contiguous.
    return k


# ----------------------------------------------------------------------------
# CPU-side prep
# ----------------------------------------------------------------------------
def _prep(edge_index):
    src = np.asarray(edge_index[0], dtype=np.int64)
    dst = np.asarray(edge_index[1], dtype=np.int64)

    deg_in = np.bincount(dst, minlength=N)
    dinv = (1.0 / np.sqrt((deg_in + 1).astype(np.float32))).astype(np.float32)

    order = np.argsort(-deg_in, kind="stable")
    ranks = np.empty(N, np.int64)
    ranks[order] = np.arange(N)
    node_core = ranks % M
    k_local = ranks // M
    node_row = node_core * SH + _row_in_shard(k_local)

    r_src = node_row[src]
    is_hi = r_src >= VIEW_A[1]                    # stream: 0=lo, 1=hi
    kd = k_local[dst]
    core_e = node_core[dst]

    # per-dst slot need: max edge count over cores, per stream
    cnt = np.zeros((M, 2, COLS), np.int64)
    np.add.at(cnt, (core_e, is_hi.astype(np.int64), kd), 1)
    need = cnt.max(axis=0)                        # [2, COLS]

    # ---- column packing (continuous, per stream): khat consecutive dsts x
    # s slots, s = max need in the run, khat*s <= 128
    cols_meta = [[], []]    # per stream: (kd0, khat, s)
    dst_col = np.full((2, COLS), -1, np.int64)
    dst_stripe = np.zeros((2, COLS), np.int64)
    dst_s = np.zeros((2, COLS), np.int64)
    for v in range(2):
        p = 0
        while p < COLS:
            if need[v, p] == 0:
                p += 1
                continue
            s = int(need[v, p])
            khat = 1
            while p + khat < COLS and need[v, p + khat] > 0:
                s2 = max(s, int(need[v, p + khat]))
                if (khat + 1) * s2 <= 128:
                    khat += 1
                    s = s2
                else:
                    break
            ci = len(cols_meta[v])
            cols_meta[v].append((p, khat, s))
            for i in range(khat):
                dst_col[v, p + i] = ci
                dst_stripe[v, p + i] = i
                dst_s[v, p + i] = s
            p += khat

    # ---- call plan: all lo calls, then all hi calls (z write, then add)
    calls = []              # (view, col_start(global), ncols, [(kd0,khat,s)])
    ccur = 0
    col_gbase = [np.full(len(cols_meta[v]), -1, np.int64) for v in range(2)]
    for v in range(2):
        i = 0
        ncv = len(cols_meta[v])
        while i < ncv:
            nc_ = min(CALL_COLS, ncv - i)
            for j in range(nc_):
                col_gbase[v][i + j] = ccur + j
            calls.append((v, ccur, nc_, cols_meta[v][i:i + nc_]))
            ccur += nc_
            i += nc_
    NCOL = ccur
    IDXTOT = NCOL * 128

    # ---- per-edge slot assignment (vectorized)
    key = (((core_e * 2) + is_hi) * COLS + kd)
    order_e = np.lexsort((r_src, key))
    key_s = key[order_e]
    uniq, start_idx, counts = np.unique(key_s, return_index=True,
                                        return_counts=True)
    pos = np.arange(E) - np.repeat(start_idx, counts)

    kd_s = kd[order_e]
    v_s = is_hi[order_e].astype(np.int64)
    s_e = dst_s[v_s, kd_s]
    assert (pos < s_e).all(), "dst stream count exceeds slot count"
    cid = dst_col[v_s, kd_s]
    assert (cid >= 0).all()
    gc = np.empty(E, np.int64)
    for v in range(2):
        m = v_s == v
        gc[m] = col_gbase[v][cid[m]]
    assert (gc >= 0).all()
    flat_pos = gc * 128 + dst_stripe[v_s, kd_s] * s_e + pos

    view_base = np.where(v_s == 1, VIEW_B[0], VIEW_A[0])
    val = r_src[order_e] - view_base
    assert val.min() >= 0 and val.max() <= 32767

    # zero rows (nodes k>=NPC): spread pad reads over all of them
    zrows = []
    for c in range(M):
        for kk2 in range(NPC, SH):
            zrows.append(c * SH + _row_in_shard(kk2))
    zrows = np.array(sorted(zrows), np.int64)
    zr_a = zrows[(zrows >= VIEW_A[0]) & (zrows < VIEW_A[1])] - VIEW_A[0]
    zr_b = zrows[(zrows >= VIEW_B[0]) & (zrows < VIEW_B[1])] - VIEW_B[0]
    assert len(zr_a) > 100 and len(zr_b) > 100

    idx_flat = np.empty((M, IDXTOT), np.int16)
    pad_stream = np.empty(IDXTOT, np.int16)
    rr = np.arange(IDXTOT)
    for (v, cs, nc_, cols_gv) in calls:
        zr = zr_a if v == 0 else zr_b
        sl = slice(cs * 128, (cs + nc_) * 128)
        pad_stream[sl] = zr[rr[sl] % len(zr)].astype(np.int16)
    idx_flat[:] = pad_stream[None, :]
    idx_flat[core_e[order_e], flat_pos] = val.astype(np.int16)

    idx_wrapped = np.empty((M, 128, IDXTOT // 16), np.int16)
    for c in range(M):
        w = idx_flat[c].reshape(IDXTOT // 16, 16).T
        idx_wrapped[c] = np.tile(w, (8, 1))

    node_of = np.full((M, COLS), -1, np.int64)
    dinv_cols = np.zeros((M, COLS), np.float32)
    for c in range(M):
        nodes_c = order[c::M]
        node_of[c, :NPC] = nodes_c
        dinv_cols[c, :NPC] = dinv[nodes_c]

    return dict(
        node_of=node_of, dinv_cols=dinv_cols, idx_wrapped=idx_wrapped,
        idx_flat=idx_flat, calls=calls, cols_meta=cols_meta,
        NCOL=NCOL, IDXTOT=IDXTOT,
    )


# ----------------------------------------------------------------------------
# Bass/Tile program
# ----------------------------------------------------------------------------
def _build(prep, reps=1):
    import concourse.bass as bass
    import concourse.tile as tile
    from concourse import bacc, mybir

    calls = prep["calls"]
    IDXTOT = prep["IDXTOT"]
    f32 = mybir.dt.float32
    bf16 = mybir.dt.bfloat16

    single = bool(os.environ.get("GNN_SINGLE"))
    nq = int(os.environ.get("GNN_NQ", "4"))
    nc = bacc.Bacc("TRN2", target_bir_lowering=False, debug=False,
                   num_devices=1 if single else M,
                   num_swdge_queues=max(nq, 1))

    x_in = nc.dram_tensor("x_fm", [128, COLS], f32, kind="ExternalInput")
    idx_in = nc.dram_tensor("idx_in", [128, IDXTOT // 16], mybir.dt.int16,
                            kind="ExternalInput")
    dinv_in = nc.dram_tensor("dinv_in", [128, COLS], f32, kind="ExternalInput")
    iden_in = nc.dram_tensor("iden_in", [128, 128], f32, kind="ExternalInput")
    idnb_in = nc.dram_tensor("idnb_in", [128, 128], bf16, kind="ExternalInput")
    W_in, B_in = [], []
    for k, (di, do) in enumerate(LAYER_DIMS):
        W_in.append(nc.dram_tensor(f"w{k+1}", [di, do], f32, kind="ExternalInput"))
        B_in.append(nc.dram_tensor(f"b{k+1}", [128, 1], f32, kind="ExternalInput"))
    Wl_in = nc.dram_tensor("wl", [H, C_OUT], f32, kind="ExternalInput")
    bl_in = nc.dram_tensor("bl", [C_OUT, 1], f32, kind="ExternalInput")
    out_t = nc.dram_tensor("out_fm", [C_OUT, COLS], f32, kind="ExternalOutput")
    dbg = bool(os.environ.get("GNN_DEBUG_Z1"))
    z1_t = (nc.dram_tensor("z1_dbg", [128, COLS], f32, kind="ExternalOutput")
            if dbg else None)

    ag_in, table = [], []
    for k in range(4):
        ag_in.append(nc.dram_tensor(f"ag_in{k+1}", [SH, 128], bf16,
                                    kind="Internal"))
        table.append(nc.dram_tensor(f"table{k+1}", [TBL, 128], bf16,
                                    kind="Internal", addr_space="Shared"))

    with tile.TileContext(nc) as tc:
        import contextlib
        with contextlib.ExitStack() as ctx:
            const_p = ctx.enter_context(tc.tile_pool(name="const", bufs=1))
            y_p = ctx.enter_context(tc.tile_pool(name="y", bufs=2))
            scr_p = ctx.enter_context(tc.tile_pool(name="scr", bufs=2))
            g_p = ctx.enter_context(tc.tile_pool(name="g", bufs=1))
            gnm_p = ctx.enter_context(tc.tile_pool(name="gnm", bufs=1))
            gbuf_p = ctx.enter_context(tc.tile_pool(name="gbuf", bufs=7))
            zh_p = ctx.enter_context(tc.tile_pool(name="zh", bufs=2))
            pmm_p = ctx.enter_context(tc.tile_pool(name="pmm", bufs=2, space="PSUM"))
            ptr_p = ctx.enter_context(tc.tile_pool(name="ptr", bufs=2, space="PSUM"))
            ptg_p = ctx.enter_context(tc.tile_pool(name="ptg", bufs=4, space="PSUM"))

            idx_t = const_p.tile([128, IDXTOT // 16], mybir.dt.int16)
            nc.sync.dma_start(idx_t[:, :], idx_in[:, :])
            dinv_t = const_p.tile([128, COLS], f32)
            nc.sync.dma_start(dinv_t[:, :], dinv_in[:, :])
            iden_t = const_p.tile([128, 128], f32)
            nc.sync.dma_start(iden_t[:, :], iden_in[:, :])
            idnb_t = const_p.tile([128, 128], bf16)
            nc.sync.dma_start(idnb_t[:, :], idnb_in[:, :])
            W_t, B_t = [], []
            for k, (di, do) in enumerate(LAYER_DIMS):
                w = const_p.tile([di, do], f32, tag=f"w{k}")
                nc.sync.dma_start(w[:, :], W_in[k][:, :])
                W_t.append(w)
                b = const_p.tile([128, 1], f32, tag=f"b{k}")
                nc.sync.dma_start(b[:, :], B_in[k][:, :])
                B_t.append(b)
            wl_t = const_p.tile([H, C_OUT], f32, tag="wl")
            nc.sync.dma_start(wl_t[:, :], Wl_in[:, :])
            bl_t = const_p.tile([C_OUT, 1], f32, tag="bl")
            nc.sync.dma_start(bl_t[:, :], bl_in[:, :])
            x_t = y_p.tile([128, COLS], f32, tag="y")
            nc.sync.dma_start(x_t[:, :], x_in[:, :])

            skip_gather = os.environ.get("GNN_SKIP_GATHER")
            skip_mm = os.environ.get("GNN_SKIP_MM")

            for rep in range(reps):
                y_prev = None
                for k, (di, do) in enumerate(LAYER_DIMS):
                    # ---- x~ = dinv * y
                    xt = scr_p.tile([128, COLS], f32, tag="scr")
                    src_t = x_t if k == 0 else y_prev
                    nc.vector.tensor_mul(xt[0:di, :], src_t[0:di, :],
                                         dinv_t[0:di, :])
                    # ---- g = W^T @ x~  (feature-major [do, COLS])
                    g_fm = g_p.tile([128, COLS], f32, tag="g")
                    for n in range(COLS // 512):
                        pm = pmm_p.tile([128, 512], f32, tag="pmm")
                        nc.tensor.matmul(pm[0:do, :], W_t[k][0:di, 0:do],
                                         xt[0:di, n * 512:(n + 1) * 512])
                        nc.scalar.copy(g_fm[0:do, n * 512:(n + 1) * 512],
                                       pm[0:do, :])
                    # ---- node-major bf16 shard for the table
                    g_nm = gnm_p.tile([128, TILES, 128], bf16, tag="gnm")
                    if do < 128:
                        nc.vector.memset(g_nm[:, :, do:128], 0.0)
                    for t in range(TILES):
                        ps = ptr_p.tile([128, 128], f32, tag="ptr")
                        nc.tensor.transpose(ps[:, 0:do],
                                            g_fm[0:do, t * 128:(t + 1) * 128],
                                            iden_t[0:do, 0:do])
                        nc.scalar.copy(g_nm[:, t, 0:do], ps[:, 0:do])
                    # ---- publish + all-gather
                    nc.sync.dma_start(
                        ag_in[k][:, :].rearrange("(t p) f -> p t f", p=128),
                        g_nm[:, :, :])
                    if single:
                        nc.sync.dma_start(table[k][0:SH, :], ag_in[k][:, :])
                    elif not os.environ.get("GNN_SKIP_AG"):
                        nc.gpsimd.collective_compute(
                            "AllGather", mybir.AluOpType.bypass,
                            replica_groups=[list(range(M))],
                            ins=[ag_in[k][:, :]], outs=[table[k][:, :]],
                        )
                    # ---- gather + transpose + segment reduce
                    z = scr_p.tile([128, COLS], f32, tag="scr")
                    if not skip_gather and not skip_mm:
                        nc.vector.memset(z[0:do, :], 0.0)
                    for call_i, (v, cs, ncols, cols) in enumerate(calls):
                        if skip_gather:
                            continue
                        view = (table[k][VIEW_A[0]:VIEW_A[1], :] if v == 0
                                else table[k][VIEW_B[0]:VIEW_B[1], :])
                        gb = gbuf_p.tile([128, CALL_COLS, 128], bf16, tag="gb")
                        nc.gpsimd.dma_gather(
                            gb[:, 0:ncols, :], view,
                            idx_t[:, cs * 8:(cs + ncols) * 8],
                            ncols * 128, ncols * 128, 128,
                            transpose=False, single_packet=False,
                            queue_num=call_i % max(nq, 1))
                        if skip_mm:
                            continue
                        for j0 in range(0, ncols, PTCOLS):
                            jn = min(PTCOLS, ncols - j0)
                            pt = ptg_p.tile([128, PTCOLS * 128], bf16,
                                            tag="ptg")
                            for j in range(jn):
                                nc.tensor.transpose(
                                    pt[:, j * 128:(j + 1) * 128],
                                    gb[:, j0 + j, 0:128],
                                    idnb_t[:, :])
                            # reduce runs of uniform (khat, s)
                            r0 = 0
                            while r0 < jn:
                                kd0, khat, s = cols[j0 + r0]
                                rn = 1
                                while (r0 + rn < jn
                                       and cols[j0 + r0 + rn][1] == khat
                                       and cols[j0 + r0 + rn][2] == s
                                       and cols[j0 + r0 + rn][0]
                                       == kd0 + rn * khat
                                       and (rn + 1) * khat <= 512):
                                    rn += 1
                                seg = pt[0:do, r0 * 128:(r0 + rn) * 128]
                                seg = seg.rearrange("p (n c) -> p n c", c=128)
                                seg = seg[:, :, 0:khat * s]
                                seg = seg.rearrange("p n (m q) -> p n m q",
                                                    q=s)
                                nd = rn * khat
                                if v == 0:
                                    zo = z[0:do, kd0:kd0 + nd].rearrange(
                                        "p (n m) -> p n m", m=khat)
                                    nc.vector.tensor_reduce(
                                        zo, seg,
                                        axis=mybir.AxisListType.X,
                                        op=mybir.AluOpType.add)
                                else:
                                    zh = zh_p.tile([128, 512], f32, tag="zh")
                                    zho = zh[0:do, 0:nd].rearrange(
                                        "p (n m) -> p n m", m=khat)
                                    nc.vector.tensor_reduce(
                                        zho, seg,
                                        axis=mybir.AxisListType.X,
                                        op=mybir.AluOpType.add)
                                    nc.vector.tensor_add(
                                        z[0:do, kd0:kd0 + nd],
                                        z[0:do, kd0:kd0 + nd],
                                        zh[0:do, 0:nd])
                                r0 += rn
                    if dbg and k == 0 and rep == 0:
                        nc.sync.dma_start(z1_t[:, :], z[:, :])
                    # ---- y = relu(dinv*(z + g) + b)
                    y = y_p.tile([128, COLS], f32, tag="y")
                    if skip_gather or skip_mm:
                        nc.vector.tensor_copy(z[0:do, :], g_fm[0:do, :])
                    else:
                        nc.vector.tensor_add(z[0:do, :], z[0:do, :],
                                             g_fm[0:do, :])
                    nc.vector.tensor_mul(z[0:do, :], z[0:do, :],
                                         dinv_t[0:do, :])
                    nc.scalar.activation(y[0:do, :], z[0:do, :],
                                         mybir.ActivationFunctionType.Relu,
                                         bias=B_t[k][0:do, :])
                    y_prev = y

                # ---- head: out = y4 @ Wl + bl
                osb = g_p.tile([C_OUT, COLS], f32, tag="g")
                for n in range(COLS // 512):
                    pm = pmm_p.tile([128, 512], f32, tag="pmm")
                    nc.tensor.matmul(pm[0:C_OUT, :], wl_t[0:H, 0:C_OUT],
                                     y_prev[0:H, n * 512:(n + 1) * 512])
                    nc.vector.tensor_scalar_add(osb[:, n * 512:(n + 1) * 512],
                                                pm[0:C_OUT, :], bl_t[:, 0:1])
                nc.sync.dma_start(out_t[:, :], osb[:, :])

    nc.compile()
    return nc


# ----------------------------------------------------------------------------
# host-side sharding glue
# ----------------------------------------------------------------------------
def _in_maps(prep, inputs):
    import ml_dtypes
    x = np.asarray(inputs["x"], np.float32)
    maps = []
    for c in range(M):
        nodes = prep["node_of"][c]
        x_fm = np.zeros((128, COLS), np.float32)
        x_fm[:, :NPC] = x[nodes[:NPC]].T
        dinv_b = np.broadcast_to(prep["dinv_cols"][c][None, :],
                                 (128, COLS)).copy()
        m = {
            "x_fm": x_fm,
            "idx_in": prep["idx_wrapped"][c],
            "dinv_in": dinv_b,
            "iden_in": np.eye(128, dtype=np.float32),
            "idnb_in": np.eye(128, dtype=np.float32).astype(ml_dtypes.bfloat16),
            "wl": np.asarray(inputs["Wl"], np.float32),
            "bl": np.asarray(inputs["bl"], np.float32).reshape(C_OUT, 1),
        }
        for k, (di, do) in enumerate(LAYER_DIMS):
            m[f"w{k+1}"] = np.asarray(inputs[f"W{k+1}"], np.float32)
            bcol = np.zeros((128, 1), np.float32)
            bcol[:do, 0] = np.asarray(inputs[f"b{k+1}"], np.float32)
            m[f"b{k+1}"] = bcol
        maps.append(m)
    return maps


def _unshard(prep, results):
    out = np.empty((N, C_OUT), np.float32)
    for c in range(M):
        o = results[c]["out_fm"]
        nodes = prep["node_of"][c]
        out[nodes[:NPC]] = o[:, :NPC].T
    return out


_CACHE = {}


def _get_program(edge_index, reps=1):
    key = (hash(edge_index.tobytes()), reps)
    if key not in _CACHE:
        prep = _prep(edge_index)
        nc = _build(prep, reps=reps)
        _CACHE[key] = (prep, nc)
    return _CACHE[key]


def kernel(**inputs):
    from concourse.bass_utils import run_bass_kernel_spmd

    edge_index = np.asarray(inputs["edge_index"], np.int32)
    reps = int(os.environ.get("GNN_REPS", "1"))
    prep, nc = _get_program(edge_index, reps)
    maps = _in_maps(prep, inputs)
    res = run_bass_kernel_spmd(nc, maps, core_ids=list(range(M)))
    kernel.last_results = res
    return _unshard(prep, res.results)


# revision 3
# speedup vs baseline: 1.0311x; 1.0311x over previous
"""GCN (4-layer message-passing + linear head) on 8 Trainium2 NeuronCores.

v4 design — non-transpose dma_gather + per-column PE transpose + DVE
segment reduce:

* Node partitioning as v2: nodes ranked by global in-degree, rank%8 = core,
  k_local = rank//8; per-node state feature-major in SBUF.
* Per layer the node-major bf16 table [40960, 128] is AllGathered as in v2.
* Messages are fetched with dma_gather(transpose=False): 256B contiguous
  rows land row i -> (partition i%128, free slot i//128) at DMA-bus speed
  (the v2 XBAR spray-transpose RX path ran ~7 ns/row).
* Edges are sorted by dst and packed into 128-row columns: per column,
  khat consecutive dsts x s slots (s = max edge count over the 8 cores for
  those dsts, khat*s <= 128); pad slots gather zero rows (spread over all
  zero rows to avoid a single-address HBM hotspot).
* Per column, one PE transpose (self-loading matmul against a bf16
  identity) lands the column feature-major in a PSUM tile (8 columns per
  bank, bf16); DVE tensor_reduce then sums each dst's contiguous s-slot
  stripe, writing z feature-major directly: lo-stream reduces write z,
  hi-stream reduces go to a scratch tile and are added.

Math (as v2): deg = indeg+1, dinv = deg^-1/2,
  y = relu(dinv*[segsum(g[src]) + g[v]] + b),  g = (dinv*x) @ W.
"""

import os
import sys
import numpy as np

for _p in ("/opt/trn_rl_repo",):
    if os.path.isdir(_p) and _p not in sys.path:
        sys.path.insert(0, _p)

# ----------------------------------------------------------------------------
# Problem constants
# ----------------------------------------------------------------------------
N = 40000
E = 640000
F_IN = 128
H = 64
C_OUT = 32
M = 8
NPC = 5000
TILES = 40
COLS = TILES * 128          # 5120
SH = COLS
TBL = SH * M                # 40960
VIEW_A = (0, 32768)
VIEW_B = (TBL - 32768, TBL)
CALL_COLS = 24              # max gather-call size in 128-row columns
PTCOLS = 16                 # transposed columns per PSUM tile (bf16)
LAYER_DIMS = [(F_IN, H), (H, 2 * H), (2 * H, 2 * H), (2 * H, H)]


def _row_in_shard(k):
    # row = k directly: high-degree (low-k) rows cluster at the shard front
    # for HBM row-buffer locality; zero rows [NPC, SH) are contiguous.
    return k


# ----------------------------------------------------------------------------
# CPU-side prep
# ----------------------------------------------------------------------------
def _prep(edge_index):
    src = np.asarray(edge_index[0], dtype=np.int64)
    dst = np.asarray(edge_index[1], dtype=np.int64)

    deg_in = np.bincount(dst, minlength=N)
    dinv = (1.0 / np.sqrt((deg_in + 1).astype(np.float32))).astype(np.float32)

    order = np.argsort(-deg_in, kind="stable")
    ranks = np.empty(N, np.int64)
    ranks[order] = np.arange(N)
    node_core = ranks % M
    k_local = ranks // M
    node_row = node_core * SH + _row_in_shard(k_local)

    r_src = node_row[src]
    is_hi = r_src >= VIEW_A[1]                    # stream: 0=lo, 1=hi
    kd = k_local[dst]
    core_e = node_core[dst]

    # per-dst slot need: max edge count over cores, per stream
    cnt = np.zeros((M, 2, COLS), np.int64)
    np.add.at(cnt, (core_e, is_hi.astype(np.int64), kd), 1)
    need = cnt.max(axis=0)                        # [2, COLS]

    # ---- column packing (continuous, per stream): khat consecutive dsts x
    # s slots, s = max need in the run, khat*s <= 128
    cols_meta = [[], []]    # per stream: (kd0, khat, s)
    dst_col = np.full((2, COLS), -1, np.int64)
    dst_stripe = np.zeros((2, COLS), np.int64)
    dst_s = np.zeros((2, COLS), np.int64)
    for v in range(2):
        p = 0
        while p < COLS:
            if need[v, p] == 0:
                p += 1
                continue
            s = int(need[v, p])
            khat = 1
            while p + khat < COLS and need[v, p + khat] > 0:
                s2 = max(s, int(need[v, p + khat]))
                if (khat + 1) * s2 <= 128:
                    khat += 1
                    s = s2
                else:
                    break
            ci = len(cols_meta[v])
            cols_meta[v].append((p, khat, s))
            for i in range(khat):
                dst_col[v, p + i] = ci
                dst_stripe[v, p + i] = i
                dst_s[v, p + i] = s
            p += khat

    # ---- call plan: all lo calls, then all hi calls (z write, then add)
    calls = []              # (view, col_start(global), ncols, [(kd0,khat,s)])
    ccur = 0
    col_gbase = [np.full(len(cols_meta[v]), -1, np.int64) for v in range(2)]
    for v in range(2):
        i = 0
        ncv = len(cols_meta[v])
        while i < ncv:
            nc_ = min(CALL_COLS, ncv - i)
            for j in range(nc_):
                col_gbase[v][i + j] = ccur + j
            calls.append((v, ccur, nc_, cols_meta[v][i:i + nc_]))
            ccur += nc_
            i += nc_
    NCOL = ccur
    IDXTOT = NCOL * 128

    # ---- per-edge slot assignment (vectorized)
    key = (((core_e * 2) + is_hi) * COLS + kd)
    order_e = np.lexsort((r_src, key))
    key_s = key[order_e]
    uniq, start_idx, counts = np.unique(key_s, return_index=True,
                                        return_counts=True)
    pos = np.arange(E) - np.repeat(start_idx, counts)

    kd_s = kd[order_e]
    v_s = is_hi[order_e].astype(np.int64)
    s_e = dst_s[v_s, kd_s]
    assert (pos < s_e).all(), "dst stream count exceeds slot count"
    cid = dst_col[v_s, kd_s]
    assert (cid >= 0).all()
    gc = np.empty(E, np.int64)
    for v in range(2):
        m = v_s == v
        gc[m] = col_gbase[v][cid[m]]
    assert (gc >= 0).all()
    flat_pos = gc * 128 + dst_stripe[v_s, kd_s] * s_e + pos

    view_base = np.where(v_s == 1, VIEW_B[0], VIEW_A[0])
    val = r_src[order_e] - view_base
    assert val.min() >= 0 and val.max() <= 32767

    # zero rows (nodes k>=NPC): spread pad reads over all of them
    zrows = []
    for c in range(M):
        for kk2 in range(NPC, SH):
            zrows.append(c * SH + _row_in_shard(kk2))
    zrows = np.array(sorted(zrows), np.int64)
    zr_a = zrows[(zrows >= VIEW_A[0]) & (zrows < VIEW_A[1])] - VIEW_A[0]
    zr_b = zrows[(zrows >= VIEW_B[0]) & (zrows < VIEW_B[1])] - VIEW_B[0]
    assert len(zr_a) > 100 and len(zr_b) > 100

    idx_flat = np.empty((M, IDXTOT), np.int16)
    pad_stream = np.empty(IDXTOT, np.int16)
    rr = np.arange(IDXTOT)
    for (v, cs, nc_, cols_gv) in calls:
        zr = zr_a if v == 0 else zr_b
        sl = slice(cs * 128, (cs + nc_) * 128)
        pad_stream[sl] = zr[rr[sl] % len(zr)].astype(np.int16)
    idx_flat[:] = pad_stream[None, :]
    idx_flat[core_e[order_e], flat_pos] = val.astype(np.int16)

    idx_wrapped = np.empty((M, 128, IDXTOT // 16), np.int16)
    for c in range(M):
        w = idx_flat[c].reshape(IDXTOT // 16, 16).T
        idx_wrapped[c] = np.tile(w, (8, 1))

    node_of = np.full((M, COLS), -1, np.int64)
    dinv_cols = np.zeros((M, COLS), np.float32)
    for c in range(M):
        nodes_c = order[c::M]
        node_of[c, :NPC] = nodes_c
        dinv_cols[c, :NPC] = dinv[nodes_c]

    return dict(
        node_of=node_of, dinv_cols=dinv_cols, idx_wrapped=idx_wrapped,
        idx_flat=idx_flat, calls=calls, cols_meta=cols_meta,
        NCOL=NCOL, IDXTOT=IDXTOT,
    )


# ----------------------------------------------------------------------------
# Bass/Tile program
# ----------------------------------------------------------------------------
def _build(prep, reps=1):
    import concourse.bass as bass
    import concourse.tile as tile
    from concourse import bacc, mybir

    calls = prep["calls"]
    IDXTOT = prep["IDXTOT"]
    f32 = mybir.dt.float32
    bf16 = mybir.dt.bfloat16

    single = bool(os.environ.get("GNN_SINGLE"))
    nq = int(os.environ.get("GNN_NQ", "4"))
    nc = bacc.Bacc("TRN2", target_bir_lowering=False, debug=False,
                   num_devices=1 if single else M,
                   num_swdge_queues=max(nq, 1))

    x_in = nc.dram_tensor("x_fm", [128, COLS], f32, kind="ExternalInput")
    idx_in = nc.dram_tensor("idx_in", [128, IDXTOT // 16], mybir.dt.int16,
                            kind="ExternalInput")
    dinv_in = nc.dram_tensor("dinv_in", [128, COLS], f32, kind="ExternalInput")
    iden_in = nc.dram_tensor("iden_in", [128, 128], f32, kind="ExternalInput")
    idnb_in = nc.dram_tensor("idnb_in", [128, 128], bf16, kind="ExternalInput")
    W_in, B_in = [], []
    for k, (di, do) in enumerate(LAYER_DIMS):
        W_in.append(nc.dram_tensor(f"w{k+1}", [di, do], f32, kind="ExternalInput"))
        B_in.append(nc.dram_tensor(f"b{k+1}", [128, 1], f32, kind="ExternalInput"))
    Wl_in = nc.dram_tensor("wl", [H, C_OUT], f32, kind="ExternalInput")
    bl_in = nc.dram_tensor("bl", [C_OUT, 1], f32, kind="ExternalInput")
    out_t = nc.dram_tensor("out_fm", [C_OUT, COLS], f32, kind="ExternalOutput")
    dbg = bool(os.environ.get("GNN_DEBUG_Z1"))
    z1_t = (nc.dram_tensor("z1_dbg", [128, COLS], f32, kind="ExternalOutput")
            if dbg else None)

    ag_in, table = [], []
    for k in range(4):
        ag_in.append(nc.dram_tensor(f"ag_in{k+1}", [SH, 128], bf16,
                                    kind="Internal"))
        table.append(nc.dram_tensor(f"table{k+1}", [TBL, 128], bf16,
                                    kind="Internal", addr_space="Shared"))

    with tile.TileContext(nc) as tc:
        import contextlib
        with contextlib.ExitStack() as ctx:
            const_p = ctx.enter_context(tc.tile_pool(name="const", bufs=1))
            y_p = ctx.enter_context(tc.tile_pool(name="y", bufs=2))
            scr_p = ctx.enter_context(tc.tile_pool(name="scr", bufs=2))
            g_p = ctx.enter_context(tc.tile_pool(name="g", bufs=1))
            gnm_p = ctx.enter_context(tc.tile_pool(name="gnm", bufs=1))
            gbuf_p = ctx.enter_context(tc.tile_pool(name="gbuf", bufs=7))
            zh_p = ctx.enter_context(tc.tile_pool(name="zh", bufs=2))
            pmm_p = ctx.enter_context(tc.tile_pool(name="pmm", bufs=2, space="PSUM"))
            ptr_p = ctx.enter_context(tc.tile_pool(name="ptr", bufs=2, space="PSUM"))
            ptg_p = ctx.enter_context(tc.tile_pool(name="ptg", bufs=2, space="PSUM"))

            idx_t = const_p.tile([128, IDXTOT // 16], mybir.dt.int16)
            nc.sync.dma_start(idx_t[:, :], idx_in[:, :])
            dinv_t = const_p.tile([128, COLS], f32)
            nc.sync.dma_start(dinv_t[:, :], dinv_in[:, :])
            iden_t = const_p.tile([128, 128], f32)
            nc.sync.dma_start(iden_t[:, :], iden_in[:, :])
            idnb_t = const_p.tile([128, 128], bf16)
            nc.sync.dma_start(idnb_t[:, :], idnb_in[:, :])
            W_t, B_t = [], []
            for k, (di, do) in enumerate(LAYER_DIMS):
                w = const_p.tile([di, do], f32, tag=f"w{k}")
                nc.sync.dma_start(w[:, :], W_in[k][:, :])
                W_t.append(w)
                b = const_p.tile([128, 1], f32, tag=f"b{k}")
                nc.sync.dma_start(b[:, :], B_in[k][:, :])
                B_t.append(b)
            wl_t = const_p.tile([H, C_OUT], f32, tag="wl")
            nc.sync.dma_start(wl_t[:, :], Wl_in[:, :])
            bl_t = const_p.tile([C_OUT, 1], f32, tag="bl")
            nc.sync.dma_start(bl_t[:, :], bl_in[:, :])
            x_t = y_p.tile([128, COLS], f32, tag="y")
            nc.sync.dma_start(x_t[:, :], x_in[:, :])

            skip_gather = os.environ.get("GNN_SKIP_GATHER")
            skip_mm = os.environ.get("GNN_SKIP_MM")

            for rep in range(reps):
                y_prev = None
                for k, (di, do) in enumerate(LAYER_DIMS):
                    # ---- x~ = dinv * y
                    xt = scr_p.tile([128, COLS], f32, tag="scr")
                    src_t = x_t if k == 0 else y_prev
                    nc.vector.tensor_mul(xt[0:di, :], src_t[0:di, :],
                                         dinv_t[0:di, :])
                    # ---- g = W^T @ x~  (feature-major [do, COLS])
                    g_fm = g_p.tile([128, COLS], f32, tag="g")
                    for n in range(COLS // 512):
                        pm = pmm_p.tile([128, 512], f32, tag="pmm")
                        nc.tensor.matmul(pm[0:do, :], W_t[k][0:di, 0:do],
                                         xt[0:di, n * 512:(n + 1) * 512])
                        nc.scalar.copy(g_fm[0:do, n * 512:(n + 1) * 512],
                                       pm[0:do, :])
                    # ---- node-major bf16 shard for the table
                    g_nm = gnm_p.tile([128, TILES, 128], bf16, tag="gnm")
                    if do < 128:
                        nc.vector.memset(g_nm[:, :, do:128], 0.0)
                    for t in range(TILES):
                        ps = ptr_p.tile([128, 128], f32, tag="ptr")
                        nc.tensor.transpose(ps[:, 0:do],
                                            g_fm[0:do, t * 128:(t + 1) * 128],
                                            iden_t[0:do, 0:do])
                        nc.scalar.copy(g_nm[:, t, 0:do], ps[:, 0:do])
                    # ---- publish + all-gather
                    nc.sync.dma_start(
                        ag_in[k][:, :].rearrange("(t p) f -> p t f", p=128),
                        g_nm[:, :, :])
                    if single:
                        nc.sync.dma_start(table[k][0:SH, :], ag_in[k][:, :])
                    elif not os.environ.get("GNN_SKIP_AG"):
                        nc.gpsimd.collective_compute(
                            "AllGather", mybir.AluOpType.bypass,
                            replica_groups=[list(range(M))],
                            ins=[ag_in[k][:, :]], outs=[table[k][:, :]],
                        )
                    # ---- gather + transpose + segment reduce
                    z = scr_p.tile([128, COLS], f32, tag="scr")
                    if not skip_gather and not skip_mm:
                        nc.vector.memset(z[0:do, :], 0.0)
                    for call_i, (v, cs, ncols, cols) in enumerate(calls):
                        if skip_gather:
                            continue
                        view = (table[k][VIEW_A[0]:VIEW_A[1], :] if v == 0
                                else table[k][VIEW_B[0]:VIEW_B[1], :])
                        gb = gbuf_p.tile([128, CALL_COLS, 128], bf16, tag="gb")
                        nc.gpsimd.dma_gather(
                            gb[:, 0:ncols, :], view,
                            idx_t[:, cs * 8:(cs + ncols) * 8],
                            ncols * 128, ncols * 128, 128,
                            transpose=False, single_packet=False,
                            queue_num=call_i % max(nq, 1))
                        if skip_mm:
                            continue
                        for j0 in range(0, ncols, PTCOLS):
                            jn = min(PTCOLS, ncols - j0)
                            pt = ptg_p.tile([128, PTCOLS * 128], bf16,
                                            tag="ptg")
                            for j in range(jn):
                                nc.tensor.transpose(
                                    pt[:, j * 128:(j + 1) * 128],
                                    gb[:, j0 + j, 0:128],
                                    idnb_t[:, :])
                            # reduce runs of uniform (khat, s)
                            r0 = 0
                            while r0 < jn:
                                kd0, khat, s = cols[j0 + r0]
                                rn = 1
                                while (r0 + rn < jn
                                       and cols[j0 + r0 + rn][1] == khat
                                       and cols[j0 + r0 + rn][2] == s
                                       and cols[j0 + r0 + rn][0]
                                       == kd0 + rn * khat
                                       and (rn + 1) * khat <= 512):
                                    rn += 1
                                seg = pt[0:do, r0 * 128:(r0 + rn) * 128]
                                seg = seg.rearrange("p (n c) -> p n c", c=128)
                                seg = seg[:, :, 0:khat * s]
                                seg = seg.rearrange("p n (m q) -> p n m q",
                                                    q=s)
                                nd = rn * khat
                                if v == 0:
                                    zo = z[0:do, kd0:kd0 + nd].rearrange(
                                        "p (n m) -> p n m", m=khat)
                                    nc.vector.tensor_reduce(
                                        zo, seg,
                                        axis=mybir.AxisListType.X,
                                        op=mybir.AluOpType.add)
                                else:
                                    zh = zh_p.tile([128, 512], f32, tag="zh")
                                    zho = zh[0:do, 0:nd].rearrange(
                                        "p (n m) -> p n m", m=khat)
                                    nc.vector.tensor_reduce(
                                        zho, seg,
                                        axis=mybir.AxisListType.X,
                                        op=mybir.AluOpType.add)
                                    nc.vector.tensor_add(
                                        z[0:do, kd0:kd0 + nd],
                                        z[0:do, kd0:kd0 + nd],
                                        zh[0:do, 0:nd])
                                r0 += rn
                    if dbg and k == 0 and rep == 0:
                        nc.sync.dma_start(z1_t[:, :], z[:, :])
                    # ---- y = relu(dinv*(z + g) + b)
                    y = y_p.tile([128, COLS], f32, tag="y")
                    if skip_gather or skip_mm:
                        nc.vector.tensor_copy(z[0:do, :], g_fm[0:do, :])
                    else:
                        nc.vector.tensor_add(z[0:do, :], z[0:do, :],
                                             g_fm[0:do, :])
                    nc.vector.tensor_mul(z[0:do, :], z[0:do, :],
                                         dinv_t[0:do, :])
                    nc.scalar.activation(y[0:do, :], z[0:do, :],
                                         mybir.ActivationFunctionType.Relu,
                                         bias=B_t[k][0:do, :])
                    y_prev = y

                # ---- head: out = y4 @ Wl + bl
                osb = g_p.tile([C_OUT, COLS], f32, tag="g")
                for n in range(COLS // 512):
                    pm = pmm_p.tile([128, 512], f32, tag="pmm")
                    nc.tensor.matmul(pm[0:C_OUT, :], wl_t[0:H, 0:C_OUT],
                                     y_prev[0:H, n * 512:(n + 1) * 512])
                    nc.vector.tensor_scalar_add(osb[:, n * 512:(n + 1) * 512],
                                                pm[0:C_OUT, :], bl_t[:, 0:1])
                nc.sync.dma_start(out_t[:, :], osb[:, :])

    nc.compile()
    return nc


# ----------------------------------------------------------------------------
# host-side sharding glue
# ----------------------------------------------------------------------------
def _in_maps(prep, inputs):
    import ml_dtypes
    x = np.asarray(inputs["x"], np.float32)
    maps = []
    for c in range(M):
        nodes = prep["node_of"][c]
        x_fm = np.zeros((128, COLS), np.float32)
        x_fm[:, :NPC] = x[nodes[:NPC]].T
        dinv_b = np.broadcast_to(prep["dinv_cols"][c][None, :],
                                 (128, COLS)).copy()
        m = {
            "x_fm": x_fm,
            "idx_in": prep["idx_wrapped"][c],
            "dinv_in": dinv_b,
            "iden_in": np.eye(128, dtype=np.float32),
            "idnb_in": np.eye(128, dtype=np.float32).astype(ml_dtypes.bfloat16),
            "wl": np.asarray(inputs["Wl"], np.float32),
            "bl": np.asarray(inputs["bl"], np.float32).reshape(C_OUT, 1),
        }
        for k, (di, do) in enumerate(LAYER_DIMS):
            m[f"w{k+1}"] = np.asarray(inputs[f"W{k+1}"], np.float32)
            bcol = np.zeros((128, 1), np.float32)
            bcol[:do, 0] = np.asarray(inputs[f"b{k+1}"], np.float32)
            m[f"b{k+1}"] = bcol
        maps.append(m)
    return maps


def _unshard(prep, results):
    out = np.empty((N, C_OUT), np.float32)
    for c in range(M):
        o = results[c]["out_fm"]
        nodes = prep["node_of"][c]
        out[nodes[:NPC]] = o[:, :NPC].T
    return out


_CACHE = {}


def _get_program(edge_index, reps=1):
    key = (hash(edge_index.tobytes()), reps)
    if key not in _CACHE:
        prep = _prep(edge_index)
        nc = _build(prep, reps=reps)
        _CACHE[key] = (prep, nc)
    return _CACHE[key]


def kernel(**inputs):
    from concourse.bass_utils import run_bass_kernel_spmd

    edge_index = np.asarray(inputs["edge_index"], np.int32)
    reps = int(os.environ.get("GNN_REPS", "1"))
    prep, nc = _get_program(edge_index, reps)
    maps = _in_maps(prep, inputs)
    res = run_bass_kernel_spmd(nc, maps, core_ids=list(range(M)))
    kernel.last_results = res
    return _unshard(prep, res.results)


# revision 4
# speedup vs baseline: 1.0877x; 1.0548x over previous
"""GCN (4-layer message-passing + linear head) on 8 Trainium2 NeuronCores.

v4 design — non-transpose dma_gather + per-column PE transpose + DVE
segment reduce:

* Node partitioning as v2: nodes ranked by global in-degree, rank%8 = core,
  k_local = rank//8; per-node state feature-major in SBUF.
* Per layer the node-major bf16 table [40960, 128] is AllGathered as in v2.
* Messages are fetched with dma_gather(transpose=False): 256B contiguous
  rows land row i -> (partition i%128, free slot i//128) at DMA-bus speed
  (the v2 XBAR spray-transpose RX path ran ~7 ns/row).
* Edges are sorted by dst and packed into 128-row columns: per column,
  khat consecutive dsts x s slots (s = max edge count over the 8 cores for
  those dsts, khat*s <= 128); pad slots gather zero rows (spread over all
  zero rows to avoid a single-address HBM hotspot).
* Per column, one PE transpose (self-loading matmul against a bf16
  identity) lands the column feature-major in a PSUM tile (8 columns per
  bank, bf16); DVE tensor_reduce then sums each dst's contiguous s-slot
  stripe, writing z feature-major directly: lo-stream reduces write z,
  hi-stream reduces go to a scratch tile and are added.

Math (as v2): deg = indeg+1, dinv = deg^-1/2,
  y = relu(dinv*[segsum(g[src]) + g[v]] + b),  g = (dinv*x) @ W.
"""

import os
import sys
import numpy as np

for _p in ("/opt/trn_rl_repo",):
    if os.path.isdir(_p) and _p not in sys.path:
        sys.path.insert(0, _p)

# ----------------------------------------------------------------------------
# Problem constants
# ----------------------------------------------------------------------------
N = 40000
E = 640000
F_IN = 128
H = 64
C_OUT = 32
M = 8
NPC = 5000
TILES = 40
COLS = TILES * 128          # 5120
SH = COLS
TBL = SH * M                # 40960
VIEW_A = (0, 32768)
VIEW_B = (TBL - 32768, TBL)
CALL_COLS = 24              # max gather-call size in 128-row columns
PTCOLS = 8                  # transposed columns per PSUM tile (bf16)
LAYER_DIMS = [(F_IN, H), (H, 2 * H), (2 * H, 2 * H), (2 * H, H)]


def _row_in_shard(k):
    # row = k directly: high-degree (low-k) rows cluster at the shard front
    # for HBM row-buffer locality; zero rows [NPC, SH) are contiguous.
    return k


# ----------------------------------------------------------------------------
# CPU-side prep
# ----------------------------------------------------------------------------
def _prep(edge_index):
    src = np.asarray(edge_index[0], dtype=np.int64)
    dst = np.asarray(edge_index[1], dtype=np.int64)

    deg_in = np.bincount(dst, minlength=N)
    dinv = (1.0 / np.sqrt((deg_in + 1).astype(np.float32))).astype(np.float32)

    order = np.argsort(-deg_in, kind="stable")
    ranks = np.empty(N, np.int64)
    ranks[order] = np.arange(N)
    node_core = ranks % M
    k_local = ranks // M
    node_row = node_core * SH + _row_in_shard(k_local)

    r_src = node_row[src]
    is_hi = r_src >= VIEW_A[1]                    # stream: 0=lo, 1=hi
    kd = k_local[dst]
    core_e = node_core[dst]

    # per-dst slot need: max edge count over cores, per stream
    cnt = np.zeros((M, 2, COLS), np.int64)
    np.add.at(cnt, (core_e, is_hi.astype(np.int64), kd), 1)
    need = cnt.max(axis=0)                        # [2, COLS]

    # ---- column packing (continuous, per stream): khat consecutive dsts x
    # s slots, s = max need in the run, khat*s <= 128
    cols_meta = [[], []]    # per stream: (kd0, khat, s)
    dst_col = np.full((2, COLS), -1, np.int64)
    dst_stripe = np.zeros((2, COLS), np.int64)
    dst_s = np.zeros((2, COLS), np.int64)
    for v in range(2):
        p = 0
        while p < COLS:
            if need[v, p] == 0:
                p += 1
                continue
            s = int(need[v, p])
            khat = 1
            while p + khat < COLS and need[v, p + khat] > 0:
                s2 = max(s, int(need[v, p + khat]))
                if (khat + 1) * s2 <= 128:
                    khat += 1
                    s = s2
                else:
                    break
            ci = len(cols_meta[v])
            cols_meta[v].append((p, khat, s))
            for i in range(khat):
                dst_col[v, p + i] = ci
                dst_stripe[v, p + i] = i
                dst_s[v, p + i] = s
            p += khat

    # ---- call plan: all lo calls, then all hi calls (z write, then add)
    calls = []              # (view, col_start(global), ncols, [(kd0,khat,s)])
    ccur = 0
    col_gbase = [np.full(len(cols_meta[v]), -1, np.int64) for v in range(2)]
    for v in range(2):
        i = 0
        ncv = len(cols_meta[v])
        while i < ncv:
            nc_ = min(CALL_COLS, ncv - i)
            for j in range(nc_):
                col_gbase[v][i + j] = ccur + j
            calls.append((v, ccur, nc_, cols_meta[v][i:i + nc_]))
            ccur += nc_
            i += nc_
    NCOL = ccur
    IDXTOT = NCOL * 128

    # ---- per-edge slot assignment (vectorized)
    key = (((core_e * 2) + is_hi) * COLS + kd)
    order_e = np.lexsort((r_src, key))
    key_s = key[order_e]
    uniq, start_idx, counts = np.unique(key_s, return_index=True,
                                        return_counts=True)
    pos = np.arange(E) - np.repeat(start_idx, counts)

    kd_s = kd[order_e]
    v_s = is_hi[order_e].astype(np.int64)
    s_e = dst_s[v_s, kd_s]
    assert (pos < s_e).all(), "dst stream count exceeds slot count"
    cid = dst_col[v_s, kd_s]
    assert (cid >= 0).all()
    gc = np.empty(E, np.int64)
    for v in range(2):
        m = v_s == v
        gc[m] = col_gbase[v][cid[m]]
    assert (gc >= 0).all()
    flat_pos = gc * 128 + dst_stripe[v_s, kd_s] * s_e + pos

    view_base = np.where(v_s == 1, VIEW_B[0], VIEW_A[0])
    val = r_src[order_e] - view_base
    assert val.min() >= 0 and val.max() <= 32767

    # zero rows (nodes k>=NPC): spread pad reads over all of them
    zrows = []
    for c in range(M):
        for kk2 in range(NPC, SH):
            zrows.append(c * SH + _row_in_shard(kk2))
    zrows = np.array(sorted(zrows), np.int64)
    zr_a = zrows[(zrows >= VIEW_A[0]) & (zrows < VIEW_A[1])] - VIEW_A[0]
    zr_b = zrows[(zrows >= VIEW_B[0]) & (zrows < VIEW_B[1])] - VIEW_B[0]
    assert len(zr_a) > 100 and len(zr_b) > 100

    idx_flat = np.empty((M, IDXTOT), np.int16)
    pad_stream = np.empty(IDXTOT, np.int16)
    rr = np.arange(IDXTOT)
    for (v, cs, nc_, cols_gv) in calls:
        zr = zr_a if v == 0 else zr_b
        sl = slice(cs * 128, (cs + nc_) * 128)
        pad_stream[sl] = zr[rr[sl] % len(zr)].astype(np.int16)
    idx_flat[:] = pad_stream[None, :]
    idx_flat[core_e[order_e], flat_pos] = val.astype(np.int16)

    idx_wrapped = np.empty((M, 128, IDXTOT // 16), np.int16)
    for c in range(M):
        w = idx_flat[c].reshape(IDXTOT // 16, 16).T
        idx_wrapped[c] = np.tile(w, (8, 1))

    node_of = np.full((M, COLS), -1, np.int64)
    dinv_cols = np.zeros((M, COLS), np.float32)
    for c in range(M):
        nodes_c = order[c::M]
        node_of[c, :NPC] = nodes_c
        dinv_cols[c, :NPC] = dinv[nodes_c]

    return dict(
        node_of=node_of, dinv_cols=dinv_cols, idx_wrapped=idx_wrapped,
        idx_flat=idx_flat, calls=calls, cols_meta=cols_meta,
        NCOL=NCOL, IDXTOT=IDXTOT,
    )


# ----------------------------------------------------------------------------
# Bass/Tile program
# ----------------------------------------------------------------------------
def _build(prep, reps=1):
    import concourse.bass as bass
    import concourse.tile as tile
    from concourse import bacc, mybir

    calls = prep["calls"]
    IDXTOT = prep["IDXTOT"]
    f32 = mybir.dt.float32
    bf16 = mybir.dt.bfloat16

    single = bool(os.environ.get("GNN_SINGLE"))
    nq = int(os.environ.get("GNN_NQ", "4"))
    nc = bacc.Bacc("TRN2", target_bir_lowering=False, debug=False,
                   num_devices=1 if single else M,
                   num_swdge_queues=max(nq, 1))

    x_in = nc.dram_tensor("x_fm", [128, COLS], f32, kind="ExternalInput")
    idx_in = nc.dram_tensor("idx_in", [128, IDXTOT // 16], mybir.dt.int16,
                            kind="ExternalInput")
    dinv_in = nc.dram_tensor("dinv_in", [128, COLS], f32, kind="ExternalInput")
    iden_in = nc.dram_tensor("iden_in", [128, 128], f32, kind="ExternalInput")
    idnb_in = nc.dram_tensor("idnb_in", [128, 128], bf16, kind="ExternalInput")
    W_in, B_in = [], []
    for k, (di, do) in enumerate(LAYER_DIMS):
        W_in.append(nc.dram_tensor(f"w{k+1}", [di, do], f32, kind="ExternalInput"))
        B_in.append(nc.dram_tensor(f"b{k+1}", [128, 1], f32, kind="ExternalInput"))
    Wl_in = nc.dram_tensor("wl", [H, C_OUT], f32, kind="ExternalInput")
    bl_in = nc.dram_tensor("bl", [C_OUT, 1], f32, kind="ExternalInput")
    out_t = nc.dram_tensor("out_fm", [C_OUT, COLS], f32, kind="ExternalOutput")
    dbg = bool(os.environ.get("GNN_DEBUG_Z1"))
    z1_t = (nc.dram_tensor("z1_dbg", [128, COLS], f32, kind="ExternalOutput")
            if dbg else None)

    ag_in, table = [], []
    for k in range(4):
        ag_in.append(nc.dram_tensor(f"ag_in{k+1}", [SH, 128], bf16,
                                    kind="Internal"))
        table.append(nc.dram_tensor(f"table{k+1}", [TBL, 128], bf16,
                                    kind="Internal", addr_space="Shared"))

    with tile.TileContext(nc) as tc:
        import contextlib
        with contextlib.ExitStack() as ctx:
            const_p = ctx.enter_context(tc.tile_pool(name="const", bufs=1))
            y_p = ctx.enter_context(tc.tile_pool(name="y", bufs=2))
            scr_p = ctx.enter_context(tc.tile_pool(name="scr", bufs=2))
            g_p = ctx.enter_context(tc.tile_pool(name="g", bufs=1))
            gnm_p = ctx.enter_context(tc.tile_pool(name="gnm", bufs=1))
            gbuf_p = ctx.enter_context(tc.tile_pool(name="gbuf", bufs=7))
            zh_p = ctx.enter_context(tc.tile_pool(name="zh", bufs=2))
            pmm_p = ctx.enter_context(tc.tile_pool(name="pmm", bufs=2, space="PSUM"))
            ptr_p = ctx.enter_context(tc.tile_pool(name="ptr", bufs=2, space="PSUM"))
            ptg_p = ctx.enter_context(tc.tile_pool(name="ptg", bufs=4, space="PSUM"))

            idx_t = const_p.tile([128, IDXTOT // 16], mybir.dt.int16)
            nc.sync.dma_start(idx_t[:, :], idx_in[:, :])
            dinv_t = const_p.tile([128, COLS], f32)
            nc.sync.dma_start(dinv_t[:, :], dinv_in[:, :])
            iden_t = const_p.tile([128, 128], f32)
            nc.sync.dma_start(iden_t[:, :], iden_in[:, :])
            idnb_t = const_p.tile([128, 128], bf16)
            nc.sync.dma_start(idnb_t[:, :], idnb_in[:, :])
            W_t, B_t = [], []
            for k, (di, do) in enumerate(LAYER_DIMS):
                w = const_p.tile([di, do], f32, tag=f"w{k}")
                nc.sync.dma_start(w[:, :], W_in[k][:, :])
                W_t.append(w)
                b = const_p.tile([128, 1], f32, tag=f"b{k}")
                nc.sync.dma_start(b[:, :], B_in[k][:, :])
                B_t.append(b)
            wl_t = const_p.tile([H, C_OUT], f32, tag="wl")
            nc.sync.dma_start(wl_t[:, :], Wl_in[:, :])
            bl_t = const_p.tile([C_OUT, 1], f32, tag="bl")
            nc.sync.dma_start(bl_t[:, :], bl_in[:, :])
            x_t = y_p.tile([128, COLS], f32, tag="y")
            nc.sync.dma_start(x_t[:, :], x_in[:, :])

            skip_gather = os.environ.get("GNN_SKIP_GATHER")
            skip_mm = os.environ.get("GNN_SKIP_MM")

            for rep in range(reps):
                y_prev = None
                for k, (di, do) in enumerate(LAYER_DIMS):
                    # ---- x~ = dinv * y
                    xt = scr_p.tile([128, COLS], f32, tag="scr")
                    src_t = x_t if k == 0 else y_prev
                    nc.vector.tensor_mul(xt[0:di, :], src_t[0:di, :],
                                         dinv_t[0:di, :])
                    # ---- g = W^T @ x~  (feature-major [do, COLS])
                    g_fm = g_p.tile([128, COLS], f32, tag="g")
                    for n in range(COLS // 512):
                        pm = pmm_p.tile([128, 512], f32, tag="pmm")
                        nc.tensor.matmul(pm[0:do, :], W_t[k][0:di, 0:do],
                                         xt[0:di, n * 512:(n + 1) * 512])
                        nc.scalar.copy(g_fm[0:do, n * 512:(n + 1) * 512],
                                       pm[0:do, :])
                    # ---- node-major bf16 shard for the table
                    g_nm = gnm_p.tile([128, TILES, 128], bf16, tag="gnm")
                    if do < 128:
                        nc.vector.memset(g_nm[:, :, do:128], 0.0)
                    for t in range(TILES):
                        ps = ptr_p.tile([128, 128], f32, tag="ptr")
                        nc.tensor.transpose(ps[:, 0:do],
                                            g_fm[0:do, t * 128:(t + 1) * 128],
                                            iden_t[0:do, 0:do])
                        nc.scalar.copy(g_nm[:, t, 0:do], ps[:, 0:do])
                    # ---- publish + all-gather
                    nc.sync.dma_start(
                        ag_in[k][:, :].rearrange("(t p) f -> p t f", p=128),
                        g_nm[:, :, :])
                    if single:
                        nc.sync.dma_start(table[k][0:SH, :], ag_in[k][:, :])
                    elif not os.environ.get("GNN_SKIP_AG"):
                        nc.gpsimd.collective_compute(
                            "AllGather", mybir.AluOpType.bypass,
                            replica_groups=[list(range(M))],
                            ins=[ag_in[k][:, :]], outs=[table[k][:, :]],
                        )
                    # ---- gather + transpose + segment reduce
                    z = scr_p.tile([128, COLS], f32, tag="scr")
                    if not skip_gather and not skip_mm:
                        nc.vector.memset(z[0:do, :], 0.0)
                    for call_i, (v, cs, ncols, cols) in enumerate(calls):
                        if skip_gather:
                            continue
                        view = (table[k][VIEW_A[0]:VIEW_A[1], :] if v == 0
                                else table[k][VIEW_B[0]:VIEW_B[1], :])
                        gb = gbuf_p.tile([128, CALL_COLS, 128], bf16, tag="gb")
                        nc.gpsimd.dma_gather(
                            gb[:, 0:ncols, :], view,
                            idx_t[:, cs * 8:(cs + ncols) * 8],
                            ncols * 128, ncols * 128, 128,
                            transpose=False, single_packet=False,
                            queue_num=call_i % max(nq, 1))
                        if skip_mm:
                            continue
                        for j0 in range(0, ncols, PTCOLS):
                            jn = min(PTCOLS, ncols - j0)
                            pt = ptg_p.tile([128, PTCOLS * 128], bf16,
                                            tag="ptg")
                            for j in range(jn):
                                nc.tensor.transpose(
                                    pt[:, j * 128:(j + 1) * 128],
                                    gb[:, j0 + j, 0:128],
                                    idnb_t[:, :])
                            # reduce runs of uniform (khat, s)
                            r0 = 0
                            while r0 < jn:
                                kd0, khat, s = cols[j0 + r0]
                                rn = 1
                                while (r0 + rn < jn
                                       and cols[j0 + r0 + rn][1] == khat
                                       and cols[j0 + r0 + rn][2] == s
                                       and cols[j0 + r0 + rn][0]
                                       == kd0 + rn * khat
                                       and (rn + 1) * khat <= 512):
                                    rn += 1
                                seg = pt[0:do, r0 * 128:(r0 + rn) * 128]
                                seg = seg.rearrange("p (n c) -> p n c", c=128)
                                seg = seg[:, :, 0:khat * s]
                                seg = seg.rearrange("p n (m q) -> p n m q",
                                                    q=s)
                                nd = rn * khat
                                if v == 0:
                                    zo = z[0:do, kd0:kd0 + nd].rearrange(
                                        "p (n m) -> p n m", m=khat)
                                    nc.vector.tensor_reduce(
                                        zo, seg,
                                        axis=mybir.AxisListType.X,
                                        op=mybir.AluOpType.add)
                                else:
                                    zh = zh_p.tile([128, 512], f32, tag="zh")
                                    zho = zh[0:do, 0:nd].rearrange(
                                        "p (n m) -> p n m", m=khat)
                                    nc.vector.tensor_reduce(
                                        zho, seg,
                                        axis=mybir.AxisListType.X,
                                        op=mybir.AluOpType.add)
                                    nc.vector.tensor_add(
                                        z[0:do, kd0:kd0 + nd],
                                        z[0:do, kd0:kd0 + nd],
                                        zh[0:do, 0:nd])
                                r0 += rn
                    if dbg and k == 0 and rep == 0:
                        nc.sync.dma_start(z1_t[:, :], z[:, :])
                    # ---- y = relu(dinv*(z + g) + b)
                    y = y_p.tile([128, COLS], f32, tag="y")
                    if skip_gather or skip_mm:
                        nc.vector.tensor_copy(z[0:do, :], g_fm[0:do, :])
                    else:
                        nc.vector.tensor_add(z[0:do, :], z[0:do, :],
                                             g_fm[0:do, :])
                    nc.vector.tensor_mul(z[0:do, :], z[0:do, :],
                                         dinv_t[0:do, :])
                    nc.scalar.activation(y[0:do, :], z[0:do, :],
                                         mybir.ActivationFunctionType.Relu,
                                         bias=B_t[k][0:do, :])
                    y_prev = y

                # ---- head: out = y4 @ Wl + bl
                osb = g_p.tile([C_OUT, COLS], f32, tag="g")
                for n in range(COLS // 512):
                    pm = pmm_p.tile([128, 512], f32, tag="pmm")
                    nc.tensor.matmul(pm[0:C_OUT, :], wl_t[0:H, 0:C_OUT],
                                     y_prev[0:H, n * 512:(n + 1) * 512])
                    nc.vector.tensor_scalar_add(osb[:, n * 512:(n + 1) * 512],
                                                pm[0:C_OUT, :], bl_t[:, 0:1])
                nc.sync.dma_start(out_t[:, :], osb[:, :])

    nc.compile()
    return nc


# ----------------------------------------------------------------------------
# host-side sharding glue
# ----------------------------------------------------------------------------
def _in_maps(prep, inputs):
    import ml_dtypes
    x = np.asarray(inputs["x"], np.float32)
    maps = []
    for c in range(M):
        nodes = prep["node_of"][c]
        x_fm = np.zeros((128, COLS), np.float32)
        x_fm[:, :NPC] = x[nodes[:NPC]].T
        dinv_b = np.broadcast_to(prep["dinv_cols"][c][None, :],
                                 (128, COLS)).copy()
        m = {
            "x_fm": x_fm,
            "idx_in": prep["idx_wrapped"][c],
            "dinv_in": dinv_b,
            "iden_in": np.eye(128, dtype=np.float32),
            "idnb_in": np.eye(128, dtype=np.float32).astype(ml_dtypes.bfloat16),
            "wl": np.asarray(inputs["Wl"], np.float32),
            "bl": np.asarray(inputs["bl"], np.float32).reshape(C_OUT, 1),
        }
        for k, (di, do) in enumerate(LAYER_DIMS):
            m[f"w{k+1}"] = np.asarray(inputs[f"W{k+1}"], np.float32)
            bcol = np.zeros((128, 1), np.float32)
            bcol[:do, 0] = np.asarray(inputs[f"b{k+1}"], np.float32)
            m[f"b{k+1}"] = bcol
        maps.append(m)
    return maps


def _unshard(prep, results):
    out = np.empty((N, C_OUT), np.float32)
    for c in range(M):
        o = results[c]["out_fm"]
        nodes = prep["node_of"][c]
        out[nodes[:NPC]] = o[:, :NPC].T
    return out


_CACHE = {}


def _get_program(edge_index, reps=1):
    key = (hash(edge_index.tobytes()), reps)
    if key not in _CACHE:
        prep = _prep(edge_index)
        nc = _build(prep, reps=reps)
        _CACHE[key] = (prep, nc)
    return _CACHE[key]


def kernel(**inputs):
    from concourse.bass_utils import run_bass_kernel_spmd

    edge_index = np.asarray(inputs["edge_index"], np.int32)
    reps = int(os.environ.get("GNN_REPS", "1"))
    prep, nc = _get_program(edge_index, reps)
    maps = _in_maps(prep, inputs)
    res = run_bass_kernel_spmd(nc, maps, core_ids=list(range(M)))
    kernel.last_results = res
    return _unshard(prep, res.results)


# revision 6
# speedup vs baseline: 1.1945x; 1.0982x over previous
"""GCN (4-layer message-passing + linear head) on 8 Trainium2 NeuronCores.

v4 design — non-transpose dma_gather + per-column PE transpose + DVE
segment reduce:

* Node partitioning as v2: nodes ranked by global in-degree, rank%8 = core,
  k_local = rank//8; per-node state feature-major in SBUF.
* Per layer the node-major bf16 table [40960, 128] is AllGathered as in v2.
* Messages are fetched with dma_gather(transpose=False): 256B contiguous
  rows land row i -> (partition i%128, free slot i//128) at DMA-bus speed
  (the v2 XBAR spray-transpose RX path ran ~7 ns/row).
* Edges are sorted by dst and packed into 128-row columns: per column,
  khat consecutive dsts x s slots (s = max edge count over the 8 cores for
  those dsts, khat*s <= 128); pad slots gather zero rows (spread over all
  zero rows to avoid a single-address HBM hotspot).
* Per column, one PE transpose (self-loading matmul against a bf16
  identity) lands the column feature-major in a PSUM tile (8 columns per
  bank, bf16); DVE tensor_reduce then sums each dst's contiguous s-slot
  stripe, writing z feature-major directly: lo-stream reduces write z,
  hi-stream reduces go to a scratch tile and are added.

Math (as v2): deg = indeg+1, dinv = deg^-1/2,
  y = relu(dinv*[segsum(g[src]) + g[v]] + b),  g = (dinv*x) @ W.
"""

import os
import sys
import numpy as np

for _p in ("/opt/trn_rl_repo",):
    if os.path.isdir(_p) and _p not in sys.path:
        sys.path.insert(0, _p)

# ----------------------------------------------------------------------------
# Problem constants
# ----------------------------------------------------------------------------
N = 40000
E = 640000
F_IN = 128
H = 64
C_OUT = 32
M = 8
NPC = 5000
TILES = 40
COLS = TILES * 128          # 5120
SH = COLS
TBL = SH * M                # 40960
VIEW_A = (0, 32768)
VIEW_B = (TBL - 32768, TBL)
CALL_COLS = 24              # max gather-call size in 128-row columns
PTCOLS = 8                  # transposed columns per PSUM tile (bf16)
LAYER_DIMS = [(F_IN, H), (H, 2 * H), (2 * H, 2 * H), (2 * H, H)]


def _row_in_shard(k):
    # row = k directly: high-degree (low-k) rows cluster at the shard front
    # for HBM row-buffer locality; zero rows [NPC, SH) are contiguous.
    return k


# ----------------------------------------------------------------------------
# CPU-side prep
# ----------------------------------------------------------------------------
def _prep(edge_index):
    src = np.asarray(edge_index[0], dtype=np.int64)
    dst = np.asarray(edge_index[1], dtype=np.int64)

    deg_in = np.bincount(dst, minlength=N)
    dinv = (1.0 / np.sqrt((deg_in + 1).astype(np.float32))).astype(np.float32)

    order = np.argsort(-deg_in, kind="stable")
    ranks = np.empty(N, np.int64)
    ranks[order] = np.arange(N)
    node_core = ranks % M
    k_local = ranks // M
    node_row = node_core * SH + _row_in_shard(k_local)

    r_src = node_row[src]
    is_hi = r_src >= VIEW_A[1]                    # stream: 0=lo, 1=hi
    kd = k_local[dst]
    core_e = node_core[dst]

    # per-dst slot need: max edge count over cores, per stream
    cnt = np.zeros((M, 2, COLS), np.int64)
    np.add.at(cnt, (core_e, is_hi.astype(np.int64), kd), 1)
    need = cnt.max(axis=0)                        # [2, COLS]

    # ---- column packing (continuous, per stream): khat consecutive dsts x
    # s slots, s = max need in the run, khat*s <= 128
    cols_meta = [[], []]    # per stream: (kd0, khat, s)
    dst_col = np.full((2, COLS), -1, np.int64)
    dst_stripe = np.zeros((2, COLS), np.int64)
    dst_s = np.zeros((2, COLS), np.int64)
    for v in range(2):
        p = 0
        while p < COLS:
            if need[v, p] == 0:
                p += 1
                continue
            s = int(need[v, p])
            khat = 1
            while p + khat < COLS and need[v, p + khat] > 0:
                s2 = max(s, int(need[v, p + khat]))
                if (khat + 1) * s2 <= 128:
                    khat += 1
                    s = s2
                else:
                    break
            ci = len(cols_meta[v])
            cols_meta[v].append((p, khat, s))
            for i in range(khat):
                dst_col[v, p + i] = ci
                dst_stripe[v, p + i] = i
                dst_s[v, p + i] = s
            p += khat

    # ---- call plan: lo and hi calls interleaved by dst progress (hi
    # reduces write a separate z_hi buffer, so ordering is free)
    specs = []              # (first_kd, v, local col range)
    for v in range(2):
        i = 0
        ncv = len(cols_meta[v])
        while i < ncv:
            nc_ = min(CALL_COLS, ncv - i)
            specs.append((cols_meta[v][i][0], v, i, nc_))
            i += nc_
    specs.sort()
    calls = []              # (view, col_start(global), ncols, [(kd0,khat,s)])
    ccur = 0
    col_gbase = [np.full(len(cols_meta[v]), -1, np.int64) for v in range(2)]
    for (_, v, i, nc_) in specs:
        for j in range(nc_):
            col_gbase[v][i + j] = ccur + j
        calls.append((v, ccur, nc_, cols_meta[v][i:i + nc_]))
        ccur += nc_
    NCOL = ccur
    IDXTOT = NCOL * 128

    # ---- per-edge slot assignment (vectorized)
    key = (((core_e * 2) + is_hi) * COLS + kd)
    order_e = np.lexsort((r_src, key))
    key_s = key[order_e]
    uniq, start_idx, counts = np.unique(key_s, return_index=True,
                                        return_counts=True)
    pos = np.arange(E) - np.repeat(start_idx, counts)

    kd_s = kd[order_e]
    v_s = is_hi[order_e].astype(np.int64)
    s_e = dst_s[v_s, kd_s]
    assert (pos < s_e).all(), "dst stream count exceeds slot count"
    cid = dst_col[v_s, kd_s]
    assert (cid >= 0).all()
    gc = np.empty(E, np.int64)
    for v in range(2):
        m = v_s == v
        gc[m] = col_gbase[v][cid[m]]
    assert (gc >= 0).all()
    flat_pos = gc * 128 + dst_stripe[v_s, kd_s] * s_e + pos

    view_base = np.where(v_s == 1, VIEW_B[0], VIEW_A[0])
    val = r_src[order_e] - view_base
    assert val.min() >= 0 and val.max() <= 32767

    # zero rows (nodes k>=NPC): spread pad reads over all of them
    zrows = []
    for c in range(M):
        for kk2 in range(NPC, SH):
            zrows.append(c * SH + _row_in_shard(kk2))
    zrows = np.array(sorted(zrows), np.int64)
    zr_a = zrows[(zrows >= VIEW_A[0]) & (zrows < VIEW_A[1])] - VIEW_A[0]
    zr_b = zrows[(zrows >= VIEW_B[0]) & (zrows < VIEW_B[1])] - VIEW_B[0]
    assert len(zr_a) > 100 and len(zr_b) > 100

    idx_flat = np.empty((M, IDXTOT), np.int16)
    pad_stream = np.empty(IDXTOT, np.int16)
    rr = np.arange(IDXTOT)
    for (v, cs, nc_, cols_gv) in calls:
        zr = zr_a if v == 0 else zr_b
        sl = slice(cs * 128, (cs + nc_) * 128)
        pad_stream[sl] = zr[rr[sl] % len(zr)].astype(np.int16)
    idx_flat[:] = pad_stream[None, :]
    idx_flat[core_e[order_e], flat_pos] = val.astype(np.int16)

    idx_wrapped = np.empty((M, 128, IDXTOT // 16), np.int16)
    for c in range(M):
        w = idx_flat[c].reshape(IDXTOT // 16, 16).T
        idx_wrapped[c] = np.tile(w, (8, 1))

    node_of = np.full((M, COLS), -1, np.int64)
    dinv_cols = np.zeros((M, COLS), np.float32)
    for c in range(M):
        nodes_c = order[c::M]
        node_of[c, :NPC] = nodes_c
        dinv_cols[c, :NPC] = dinv[nodes_c]

    return dict(
        node_of=node_of, dinv_cols=dinv_cols, idx_wrapped=idx_wrapped,
        idx_flat=idx_flat, calls=calls, cols_meta=cols_meta,
        NCOL=NCOL, IDXTOT=IDXTOT,
    )


# ----------------------------------------------------------------------------
# Bass/Tile program
# ----------------------------------------------------------------------------
def _build(prep, reps=1):
    import concourse.bass as bass
    import concourse.tile as tile
    from concourse import bacc, mybir

    calls = prep["calls"]
    IDXTOT = prep["IDXTOT"]
    f32 = mybir.dt.float32
    bf16 = mybir.dt.bfloat16

    single = bool(os.environ.get("GNN_SINGLE"))
    nq = int(os.environ.get("GNN_NQ", "4"))
    nc = bacc.Bacc("TRN2", target_bir_lowering=False, debug=False,
                   num_devices=1 if single else M,
                   num_swdge_queues=max(nq, 1))

    x_in = nc.dram_tensor("x_fm", [128, COLS], f32, kind="ExternalInput")
    idx_in = nc.dram_tensor("idx_in", [128, IDXTOT // 16], mybir.dt.int16,
                            kind="ExternalInput")
    dinv_in = nc.dram_tensor("dinv_in", [128, COLS], f32, kind="ExternalInput")
    iden_in = nc.dram_tensor("iden_in", [128, 128], f32, kind="ExternalInput")
    idnb_in = nc.dram_tensor("idnb_in", [128, 128], bf16, kind="ExternalInput")
    W_in, B_in = [], []
    for k, (di, do) in enumerate(LAYER_DIMS):
        W_in.append(nc.dram_tensor(f"w{k+1}", [di, do], f32, kind="ExternalInput"))
        B_in.append(nc.dram_tensor(f"b{k+1}", [128, 1], f32, kind="ExternalInput"))
    Wl_in = nc.dram_tensor("wl", [H, C_OUT], f32, kind="ExternalInput")
    bl_in = nc.dram_tensor("bl", [C_OUT, 1], f32, kind="ExternalInput")
    out_t = nc.dram_tensor("out_fm", [C_OUT, COLS], f32, kind="ExternalOutput")
    dbg = bool(os.environ.get("GNN_DEBUG_Z1"))
    z1_t = (nc.dram_tensor("z1_dbg", [128, COLS], f32, kind="ExternalOutput")
            if dbg else None)

    ag_in, table = [], []
    for k in range(4):
        ag_in.append(nc.dram_tensor(f"ag_in{k+1}", [SH, 128], bf16,
                                    kind="Internal"))
        table.append(nc.dram_tensor(f"table{k+1}", [TBL, 128], bf16,
                                    kind="Internal", addr_space="Shared"))

    with tile.TileContext(nc) as tc:
        import contextlib
        with contextlib.ExitStack() as ctx:
            const_p = ctx.enter_context(tc.tile_pool(name="const", bufs=1))
            y_p = ctx.enter_context(tc.tile_pool(name="y", bufs=2))
            scr_p = ctx.enter_context(tc.tile_pool(name="scr", bufs=2))
            g_p = ctx.enter_context(tc.tile_pool(name="g", bufs=1))
            gnm_p = ctx.enter_context(tc.tile_pool(name="gnm", bufs=1))
            gbuf_p = ctx.enter_context(tc.tile_pool(name="gbuf", bufs=7))
            zhi_p = ctx.enter_context(tc.tile_pool(name="zhi", bufs=1))
            pmm_p = ctx.enter_context(tc.tile_pool(name="pmm", bufs=2, space="PSUM"))
            ptr_p = ctx.enter_context(tc.tile_pool(name="ptr", bufs=2, space="PSUM"))
            ptg_p = ctx.enter_context(tc.tile_pool(name="ptg", bufs=4, space="PSUM"))

            idx_t = const_p.tile([128, IDXTOT // 16], mybir.dt.int16)
            nc.sync.dma_start(idx_t[:, :], idx_in[:, :])
            dinv_t = const_p.tile([128, COLS], f32)
            nc.sync.dma_start(dinv_t[:, :], dinv_in[:, :])
            iden_t = const_p.tile([128, 128], f32)
            nc.sync.dma_start(iden_t[:, :], iden_in[:, :])
            idnb_t = const_p.tile([128, 128], bf16)
            nc.sync.dma_start(idnb_t[:, :], idnb_in[:, :])
            W_t, B_t = [], []
            for k, (di, do) in enumerate(LAYER_DIMS):
                w = const_p.tile([di, do], f32, tag=f"w{k}")
                nc.sync.dma_start(w[:, :], W_in[k][:, :])
                W_t.append(w)
                b = const_p.tile([128, 1], f32, tag=f"b{k}")
                nc.sync.dma_start(b[:, :], B_in[k][:, :])
                B_t.append(b)
            wl_t = const_p.tile([H, C_OUT], f32, tag="wl")
            nc.sync.dma_start(wl_t[:, :], Wl_in[:, :])
            bl_t = const_p.tile([C_OUT, 1], f32, tag="bl")
            nc.sync.dma_start(bl_t[:, :], bl_in[:, :])
            x_t = y_p.tile([128, COLS], f32, tag="y")
            nc.sync.dma_start(x_t[:, :], x_in[:, :])

            skip_gather = os.environ.get("GNN_SKIP_GATHER")
            skip_mm = os.environ.get("GNN_SKIP_MM")

            for rep in range(reps):
                y_prev = None
                for k, (di, do) in enumerate(LAYER_DIMS):
                    # ---- x~ = dinv * y
                    xt = scr_p.tile([128, COLS], f32, tag="scr")
                    src_t = x_t if k == 0 else y_prev
                    nc.vector.tensor_mul(xt[0:di, :], src_t[0:di, :],
                                         dinv_t[0:di, :])
                    # ---- g = W^T @ x~  (feature-major [do, COLS])
                    g_fm = g_p.tile([128, COLS], f32, tag="g")
                    for n in range(COLS // 512):
                        pm = pmm_p.tile([128, 512], f32, tag="pmm")
                        nc.tensor.matmul(pm[0:do, :], W_t[k][0:di, 0:do],
                                         xt[0:di, n * 512:(n + 1) * 512])
                        nc.scalar.copy(g_fm[0:do, n * 512:(n + 1) * 512],
                                       pm[0:do, :])
                    # ---- node-major bf16 shard for the table
                    g_nm = gnm_p.tile([128, TILES, 128], bf16, tag="gnm")
                    if do < 128:
                        nc.vector.memset(g_nm[:, :, do:128], 0.0)
                    for t in range(TILES):
                        ps = ptr_p.tile([128, 128], f32, tag="ptr")
                        nc.tensor.transpose(ps[:, 0:do],
                                            g_fm[0:do, t * 128:(t + 1) * 128],
                                            iden_t[0:do, 0:do])
                        nc.scalar.copy(g_nm[:, t, 0:do], ps[:, 0:do])
                    # ---- publish + all-gather
                    nc.sync.dma_start(
                        ag_in[k][:, :].rearrange("(t p) f -> p t f", p=128),
                        g_nm[:, :, :])
                    if single:
                        nc.sync.dma_start(table[k][0:SH, :], ag_in[k][:, :])
                    elif not os.environ.get("GNN_SKIP_AG"):
                        nc.gpsimd.collective_compute(
                            "AllGather", mybir.AluOpType.bypass,
                            replica_groups=[list(range(M))],
                            ins=[ag_in[k][:, :]], outs=[table[k][:, :]],
                        )
                    # ---- gather + transpose + segment reduce
                    z = scr_p.tile([128, COLS], f32, tag="scr")
                    zhi = zhi_p.tile([128, COLS], bf16, tag="zhi")
                    if not skip_gather and not skip_mm:
                        nc.vector.memset(z[0:do, :], 0.0)
                        nc.vector.memset(zhi[0:do, :], 0.0)
                    for call_i, (v, cs, ncols, cols) in enumerate(calls):
                        if skip_gather:
                            continue
                        view = (table[k][VIEW_A[0]:VIEW_A[1], :] if v == 0
                                else table[k][VIEW_B[0]:VIEW_B[1], :])
                        gb = gbuf_p.tile([128, CALL_COLS, 128], bf16, tag="gb")
                        nc.gpsimd.dma_gather(
                            gb[:, 0:ncols, :], view,
                            idx_t[:, cs * 8:(cs + ncols) * 8],
                            ncols * 128, ncols * 128, 128,
                            transpose=False, single_packet=False,
                            queue_num=call_i % max(nq, 1))
                        if skip_mm:
                            continue
                        for j0 in range(0, ncols, PTCOLS):
                            jn = min(PTCOLS, ncols - j0)
                            pt = ptg_p.tile([128, PTCOLS * 128], bf16,
                                            tag="ptg")
                            for j in range(jn):
                                nc.tensor.transpose(
                                    pt[:, j * 128:(j + 1) * 128],
                                    gb[:, j0 + j, 0:128],
                                    idnb_t[:, :])
                            # reduce runs of uniform (khat, s)
                            r0 = 0
                            while r0 < jn:
                                kd0, khat, s = cols[j0 + r0]
                                rn = 1
                                while (r0 + rn < jn
                                       and cols[j0 + r0 + rn][1] == khat
                                       and cols[j0 + r0 + rn][2] == s
                                       and cols[j0 + r0 + rn][0]
                                       == kd0 + rn * khat
                                       and (rn + 1) * khat <= 512):
                                    rn += 1
                                seg = pt[0:do, r0 * 128:(r0 + rn) * 128]
                                seg = seg.rearrange("p (n c) -> p n c", c=128)
                                seg = seg[:, :, 0:khat * s]
                                seg = seg.rearrange("p n (m q) -> p n m q",
                                                    q=s)
                                nd = rn * khat
                                if v == 0:
                                    zo = z[0:do, kd0:kd0 + nd].rearrange(
                                        "p (n m) -> p n m", m=khat)
                                    nc.vector.tensor_reduce(
                                        zo, seg,
                                        axis=mybir.AxisListType.X,
                                        op=mybir.AluOpType.add)
                                else:
                                    zo = zhi[0:do, kd0:kd0 + nd].rearrange(
                                        "p (n m) -> p n m", m=khat)
                                    with nc.allow_low_precision(
                                            reason="hi partial sums (~3 "
                                            "values) stored bf16"):
                                        nc.vector.tensor_reduce(
                                            zo, seg,
                                            axis=mybir.AxisListType.X,
                                            op=mybir.AluOpType.add)
                                r0 += rn
                    if dbg and k == 0 and rep == 0:
                        nc.sync.dma_start(z1_t[:, :], z[:, :])
                    # ---- y = relu(dinv*(z + g) + b)
                    y = y_p.tile([128, COLS], f32, tag="y")
                    if skip_gather or skip_mm:
                        nc.vector.tensor_copy(z[0:do, :], g_fm[0:do, :])
                    else:
                        nc.vector.tensor_add(z[0:do, :], z[0:do, :],
                                             zhi[0:do, :])
                        nc.vector.tensor_add(z[0:do, :], z[0:do, :],
                                             g_fm[0:do, :])
                    nc.vector.tensor_mul(z[0:do, :], z[0:do, :],
                                         dinv_t[0:do, :])
                    nc.scalar.activation(y[0:do, :], z[0:do, :],
                                         mybir.ActivationFunctionType.Relu,
                                         bias=B_t[k][0:do, :])
                    y_prev = y

                # ---- head: out = y4 @ Wl + bl
                osb = g_p.tile([C_OUT, COLS], f32, tag="g")
                for n in range(COLS // 512):
                    pm = pmm_p.tile([128, 512], f32, tag="pmm")
                    nc.tensor.matmul(pm[0:C_OUT, :], wl_t[0:H, 0:C_OUT],
                                     y_prev[0:H, n * 512:(n + 1) * 512])
                    nc.vector.tensor_scalar_add(osb[:, n * 512:(n + 1) * 512],
                                                pm[0:C_OUT, :], bl_t[:, 0:1])
                nc.sync.dma_start(out_t[:, :], osb[:, :])

    nc.compile()
    return nc


# ----------------------------------------------------------------------------
# host-side sharding glue
# ----------------------------------------------------------------------------
def _in_maps(prep, inputs):
    import ml_dtypes
    x = np.asarray(inputs["x"], np.float32)
    maps = []
    for c in range(M):
        nodes = prep["node_of"][c]
        x_fm = np.zeros((128, COLS), np.float32)
        x_fm[:, :NPC] = x[nodes[:NPC]].T
        dinv_b = np.broadcast_to(prep["dinv_cols"][c][None, :],
                                 (128, COLS)).copy()
        m = {
            "x_fm": x_fm,
            "idx_in": prep["idx_wrapped"][c],
            "dinv_in": dinv_b,
            "iden_in": np.eye(128, dtype=np.float32),
            "idnb_in": np.eye(128, dtype=np.float32).astype(ml_dtypes.bfloat16),
            "wl": np.asarray(inputs["Wl"], np.float32),
            "bl": np.asarray(inputs["bl"], np.float32).reshape(C_OUT, 1),
        }
        for k, (di, do) in enumerate(LAYER_DIMS):
            m[f"w{k+1}"] = np.asarray(inputs[f"W{k+1}"], np.float32)
            bcol = np.zeros((128, 1), np.float32)
            bcol[:do, 0] = np.asarray(inputs[f"b{k+1}"], np.float32)
            m[f"b{k+1}"] = bcol
        maps.append(m)
    return maps


def _unshard(prep, results):
    out = np.empty((N, C_OUT), np.float32)
    for c in range(M):
        o = results[c]["out_fm"]
        nodes = prep["node_of"][c]
        out[nodes[:NPC]] = o[:, :NPC].T
    return out


_CACHE = {}


def _get_program(edge_index, reps=1):
    key = (hash(edge_index.tobytes()), reps)
    if key not in _CACHE:
        prep = _prep(edge_index)
        nc = _build(prep, reps=reps)
        _CACHE[key] = (prep, nc)
    return _CACHE[key]


def kernel(**inputs):
    from concourse.bass_utils import run_bass_kernel_spmd

    edge_index = np.asarray(inputs["edge_index"], np.int32)
    reps = int(os.environ.get("GNN_REPS", "1"))
    prep, nc = _get_program(edge_index, reps)
    maps = _in_maps(prep, inputs)
    res = run_bass_kernel_spmd(nc, maps, core_ids=list(range(M)))
    kernel.last_results = res
    return _unshard(prep, res.results)


# revision 7
# speedup vs baseline: 1.2156x; 1.0177x over previous
"""GCN (4-layer message-passing + linear head) on 8 Trainium2 NeuronCores.

v4 design — non-transpose dma_gather + per-column PE transpose + DVE
segment reduce:

* Node partitioning as v2: nodes ranked by global in-degree, rank%8 = core,
  k_local = rank//8; per-node state feature-major in SBUF.
* Per layer the node-major bf16 table [40960, 128] is AllGathered as in v2.
* Messages are fetched with dma_gather(transpose=False): 256B contiguous
  rows land row i -> (partition i%128, free slot i//128) at DMA-bus speed
  (the v2 XBAR spray-transpose RX path ran ~7 ns/row).
* Edges are sorted by dst and packed into 128-row columns: per column,
  khat consecutive dsts x s slots (s = max edge count over the 8 cores for
  those dsts, khat*s <= 128); pad slots gather zero rows (spread over all
  zero rows to avoid a single-address HBM hotspot).
* Per column, one PE transpose (self-loading matmul against a bf16
  identity) lands the column feature-major in a PSUM tile (8 columns per
  bank, bf16); DVE tensor_reduce then sums each dst's contiguous s-slot
  stripe, writing z feature-major directly: lo-stream reduces write z,
  hi-stream reduces go to a scratch tile and are added.

Math (as v2): deg = indeg+1, dinv = deg^-1/2,
  y = relu(dinv*[segsum(g[src]) + g[v]] + b),  g = (dinv*x) @ W.
"""

import os
import sys
import numpy as np

for _p in ("/opt/trn_rl_repo",):
    if os.path.isdir(_p) and _p not in sys.path:
        sys.path.insert(0, _p)

# ----------------------------------------------------------------------------
# Problem constants
# ----------------------------------------------------------------------------
N = 40000
E = 640000
F_IN = 128
H = 64
C_OUT = 32
M = 8
NPC = 5000
TILES = 40
COLS = TILES * 128          # 5120
SH = COLS
TBL = SH * M                # 40960
VIEW_A = (0, 32768)
VIEW_B = (TBL - 32768, TBL)
CALL_COLS = 24              # max gather-call size in 128-row columns
PTCOLS = 8                  # transposed columns per PSUM tile (bf16)
LAYER_DIMS = [(F_IN, H), (H, 2 * H), (2 * H, 2 * H), (2 * H, H)]


def _row_in_shard(k):
    # row = k directly: high-degree (low-k) rows cluster at the shard front
    # for HBM row-buffer locality; zero rows [NPC, SH) are contiguous.
    return k


# ----------------------------------------------------------------------------
# CPU-side prep
# ----------------------------------------------------------------------------
def _prep(edge_index):
    src = np.asarray(edge_index[0], dtype=np.int64)
    dst = np.asarray(edge_index[1], dtype=np.int64)

    deg_in = np.bincount(dst, minlength=N)
    dinv = (1.0 / np.sqrt((deg_in + 1).astype(np.float32))).astype(np.float32)

    order = np.argsort(-deg_in, kind="stable")
    ranks = np.empty(N, np.int64)
    ranks[order] = np.arange(N)
    node_core = ranks % M
    k_local = ranks // M
    node_row = node_core * SH + _row_in_shard(k_local)

    r_src = node_row[src]
    is_hi = r_src >= VIEW_A[1]                    # stream: 0=lo, 1=hi
    kd = k_local[dst]
    core_e = node_core[dst]

    # per-dst slot need: max edge count over cores, per stream
    cnt = np.zeros((M, 2, COLS), np.int64)
    np.add.at(cnt, (core_e, is_hi.astype(np.int64), kd), 1)
    need = cnt.max(axis=0)                        # [2, COLS]

    # ---- column packing (continuous, per stream): khat consecutive dsts x
    # s slots, s = max need in the run, khat*s <= 128
    cols_meta = [[], []]    # per stream: (kd0, khat, s)
    dst_col = np.full((2, COLS), -1, np.int64)
    dst_stripe = np.zeros((2, COLS), np.int64)
    dst_s = np.zeros((2, COLS), np.int64)
    for v in range(2):
        p = 0
        while p < COLS:
            if need[v, p] == 0:
                p += 1
                continue
            s = int(need[v, p])
            khat = 1
            while p + khat < COLS and need[v, p + khat] > 0:
                s2 = max(s, int(need[v, p + khat]))
                if (khat + 1) * s2 <= 128:
                    khat += 1
                    s = s2
                else:
                    break
            ci = len(cols_meta[v])
            cols_meta[v].append((p, khat, s))
            for i in range(khat):
                dst_col[v, p + i] = ci
                dst_stripe[v, p + i] = i
                dst_s[v, p + i] = s
            p += khat

    # ---- call plan: lo and hi calls interleaved by dst progress (hi
    # reduces write a separate z_hi buffer, so ordering is free)
    specs = []              # (first_kd, v, local col range)
    for v in range(2):
        i = 0
        ncv = len(cols_meta[v])
        while i < ncv:
            nc_ = min(CALL_COLS, ncv - i)
            specs.append((cols_meta[v][i][0], v, i, nc_))
            i += nc_
    specs.sort()
    calls = []              # (view, col_start(global), ncols, [(kd0,khat,s)])
    ccur = 0
    col_gbase = [np.full(len(cols_meta[v]), -1, np.int64) for v in range(2)]
    for (_, v, i, nc_) in specs:
        for j in range(nc_):
            col_gbase[v][i + j] = ccur + j
        calls.append((v, ccur, nc_, cols_meta[v][i:i + nc_]))
        ccur += nc_
    NCOL = ccur
    IDXTOT = NCOL * 128

    # ---- per-edge slot assignment (vectorized)
    key = (((core_e * 2) + is_hi) * COLS + kd)
    order_e = np.lexsort((r_src, key))
    key_s = key[order_e]
    uniq, start_idx, counts = np.unique(key_s, return_index=True,
                                        return_counts=True)
    pos = np.arange(E) - np.repeat(start_idx, counts)

    kd_s = kd[order_e]
    v_s = is_hi[order_e].astype(np.int64)
    s_e = dst_s[v_s, kd_s]
    assert (pos < s_e).all(), "dst stream count exceeds slot count"
    cid = dst_col[v_s, kd_s]
    assert (cid >= 0).all()
    gc = np.empty(E, np.int64)
    for v in range(2):
        m = v_s == v
        gc[m] = col_gbase[v][cid[m]]
    assert (gc >= 0).all()
    flat_pos = gc * 128 + dst_stripe[v_s, kd_s] * s_e + pos

    view_base = np.where(v_s == 1, VIEW_B[0], VIEW_A[0])
    val = r_src[order_e] - view_base
    assert val.min() >= 0 and val.max() <= 32767

    # zero rows (nodes k>=NPC): spread pad reads over all of them
    zrows = []
    for c in range(M):
        for kk2 in range(NPC, SH):
            zrows.append(c * SH + _row_in_shard(kk2))
    zrows = np.array(sorted(zrows), np.int64)
    zr_a = zrows[(zrows >= VIEW_A[0]) & (zrows < VIEW_A[1])] - VIEW_A[0]
    zr_b = zrows[(zrows >= VIEW_B[0]) & (zrows < VIEW_B[1])] - VIEW_B[0]
    assert len(zr_a) > 100 and len(zr_b) > 100

    idx_flat = np.empty((M, IDXTOT), np.int16)
    pad_stream = np.empty(IDXTOT, np.int16)
    rr = np.arange(IDXTOT)
    for (v, cs, nc_, cols_gv) in calls:
        zr = zr_a if v == 0 else zr_b
        sl = slice(cs * 128, (cs + nc_) * 128)
        pad_stream[sl] = zr[rr[sl] % len(zr)].astype(np.int16)
    idx_flat[:] = pad_stream[None, :]
    idx_flat[core_e[order_e], flat_pos] = val.astype(np.int16)

    idx_wrapped = np.empty((M, 128, IDXTOT // 16), np.int16)
    for c in range(M):
        w = idx_flat[c].reshape(IDXTOT // 16, 16).T
        idx_wrapped[c] = np.tile(w, (8, 1))

    node_of = np.full((M, COLS), -1, np.int64)
    dinv_cols = np.zeros((M, COLS), np.float32)
    for c in range(M):
        nodes_c = order[c::M]
        node_of[c, :NPC] = nodes_c
        dinv_cols[c, :NPC] = dinv[nodes_c]

    return dict(
        node_of=node_of, dinv_cols=dinv_cols, idx_wrapped=idx_wrapped,
        idx_flat=idx_flat, calls=calls, cols_meta=cols_meta,
        NCOL=NCOL, IDXTOT=IDXTOT,
    )


# ----------------------------------------------------------------------------
# Bass/Tile program
# ----------------------------------------------------------------------------
def _build(prep, reps=1):
    import concourse.bass as bass
    import concourse.tile as tile
    from concourse import bacc, mybir

    calls = prep["calls"]
    IDXTOT = prep["IDXTOT"]
    f32 = mybir.dt.float32
    bf16 = mybir.dt.bfloat16

    single = bool(os.environ.get("GNN_SINGLE"))
    nq = int(os.environ.get("GNN_NQ", "4"))
    nc = bacc.Bacc("TRN2", target_bir_lowering=False, debug=False,
                   num_devices=1 if single else M,
                   num_swdge_queues=max(nq, 1))

    x_in = nc.dram_tensor("x_fm", [128, COLS], f32, kind="ExternalInput")
    idx_in = nc.dram_tensor("idx_in", [128, IDXTOT // 16], mybir.dt.int16,
                            kind="ExternalInput")
    dinv_in = nc.dram_tensor("dinv_in", [128, COLS], f32, kind="ExternalInput")
    iden_in = nc.dram_tensor("iden_in", [128, 128], f32, kind="ExternalInput")
    idnb_in = nc.dram_tensor("idnb_in", [128, 128], bf16, kind="ExternalInput")
    W_in, B_in = [], []
    for k, (di, do) in enumerate(LAYER_DIMS):
        W_in.append(nc.dram_tensor(f"w{k+1}", [di, do], f32, kind="ExternalInput"))
        B_in.append(nc.dram_tensor(f"b{k+1}", [128, 1], f32, kind="ExternalInput"))
    Wl_in = nc.dram_tensor("wl", [H, C_OUT], f32, kind="ExternalInput")
    bl_in = nc.dram_tensor("bl", [C_OUT, 1], f32, kind="ExternalInput")
    out_t = nc.dram_tensor("out_fm", [C_OUT, COLS], f32, kind="ExternalOutput")
    dbg = bool(os.environ.get("GNN_DEBUG_Z1"))
    z1_t = (nc.dram_tensor("z1_dbg", [128, COLS], f32, kind="ExternalOutput")
            if dbg else None)

    ag_in, table = [], []
    for k in range(4):
        ag_in.append(nc.dram_tensor(f"ag_in{k+1}", [SH, 128], bf16,
                                    kind="Internal"))
        table.append(nc.dram_tensor(f"table{k+1}", [TBL, 128], bf16,
                                    kind="Internal", addr_space="Shared"))

    with tile.TileContext(nc) as tc:
        import contextlib
        with contextlib.ExitStack() as ctx:
            const_p = ctx.enter_context(tc.tile_pool(name="const", bufs=1))
            y_p = ctx.enter_context(tc.tile_pool(name="y", bufs=2))
            scr_p = ctx.enter_context(tc.tile_pool(name="scr", bufs=2))
            g_p = ctx.enter_context(tc.tile_pool(name="g", bufs=1))
            gnm_p = ctx.enter_context(tc.tile_pool(name="gnm", bufs=1))
            gbuf_p = ctx.enter_context(tc.tile_pool(name="gbuf", bufs=7))
            zhi_p = ctx.enter_context(tc.tile_pool(name="zhi", bufs=1))
            pmm_p = ctx.enter_context(tc.tile_pool(name="pmm", bufs=2, space="PSUM"))
            ptr_p = ctx.enter_context(tc.tile_pool(name="ptr", bufs=2, space="PSUM"))
            ptg_p = ctx.enter_context(tc.tile_pool(name="ptg", bufs=4, space="PSUM"))

            idx_t = const_p.tile([128, IDXTOT // 16], mybir.dt.int16)
            nc.sync.dma_start(idx_t[:, :], idx_in[:, :])
            dinv_t = const_p.tile([128, COLS], f32)
            nc.sync.dma_start(dinv_t[:, :], dinv_in[:, :])
            iden_t = const_p.tile([128, 128], f32)
            nc.sync.dma_start(iden_t[:, :], iden_in[:, :])
            idnb_t = const_p.tile([128, 128], bf16)
            nc.sync.dma_start(idnb_t[:, :], idnb_in[:, :])
            W_t, B_t = [], []
            for k, (di, do) in enumerate(LAYER_DIMS):
                w = const_p.tile([di, do], f32, tag=f"w{k}")
                nc.sync.dma_start(w[:, :], W_in[k][:, :])
                W_t.append(w)
                b = const_p.tile([128, 1], f32, tag=f"b{k}")
                nc.sync.dma_start(b[:, :], B_in[k][:, :])
                B_t.append(b)
            wl_t = const_p.tile([H, C_OUT], f32, tag="wl")
            nc.sync.dma_start(wl_t[:, :], Wl_in[:, :])
            bl_t = const_p.tile([C_OUT, 1], f32, tag="bl")
            nc.sync.dma_start(bl_t[:, :], bl_in[:, :])
            x_t = y_p.tile([128, COLS], f32, tag="y")
            nc.sync.dma_start(x_t[:, :], x_in[:, :])

            skip_gather = os.environ.get("GNN_SKIP_GATHER")
            skip_mm = os.environ.get("GNN_SKIP_MM")

            for rep in range(reps):
                y_prev = None
                for k, (di, do) in enumerate(LAYER_DIMS):
                    # ---- x~ = dinv * y
                    xt = scr_p.tile([128, COLS], f32, tag="scr")
                    src_t = x_t if k == 0 else y_prev
                    nc.vector.tensor_mul(xt[0:di, :], src_t[0:di, :],
                                         dinv_t[0:di, :])
                    # ---- g = W^T @ x~  (feature-major [do, COLS])
                    g_fm = g_p.tile([128, COLS], f32, tag="g")
                    for n in range(COLS // 512):
                        pm = pmm_p.tile([128, 512], f32, tag="pmm")
                        nc.tensor.matmul(pm[0:do, :], W_t[k][0:di, 0:do],
                                         xt[0:di, n * 512:(n + 1) * 512])
                        nc.scalar.copy(g_fm[0:do, n * 512:(n + 1) * 512],
                                       pm[0:do, :])
                    # ---- node-major bf16 shard for the table
                    g_nm = gnm_p.tile([128, TILES, 128], bf16, tag="gnm")
                    if do < 128:
                        nc.vector.memset(g_nm[:, :, do:128], 0.0)
                    for t in range(TILES):
                        ps = ptr_p.tile([128, 128], f32, tag="ptr")
                        nc.tensor.transpose(ps[:, 0:do],
                                            g_fm[0:do, t * 128:(t + 1) * 128],
                                            iden_t[0:do, 0:do])
                        nc.scalar.copy(g_nm[:, t, 0:do], ps[:, 0:do])
                    # ---- publish + all-gather
                    nc.sync.dma_start(
                        ag_in[k][:, :].rearrange("(t p) f -> p t f", p=128),
                        g_nm[:, :, :])
                    if single:
                        nc.sync.dma_start(table[k][0:SH, :], ag_in[k][:, :])
                    elif not os.environ.get("GNN_SKIP_AG"):
                        nc.gpsimd.collective_compute(
                            "AllGather", mybir.AluOpType.bypass,
                            replica_groups=[list(range(M))],
                            ins=[ag_in[k][:, :]], outs=[table[k][:, :]],
                        )
                    # ---- gather + transpose + segment reduce
                    z = scr_p.tile([128, COLS], f32, tag="scr")
                    zhi = zhi_p.tile([128, COLS], bf16, tag="zhi")
                    if not skip_gather and not skip_mm:
                        nc.vector.memset(z[0:do, :], 0.0)
                        nc.vector.memset(zhi[0:do, :], 0.0)
                    for call_i, (v, cs, ncols, cols) in enumerate(calls):
                        if skip_gather:
                            continue
                        view = (table[k][VIEW_A[0]:VIEW_A[1], :] if v == 0
                                else table[k][VIEW_B[0]:VIEW_B[1], :])
                        gb = gbuf_p.tile([128, CALL_COLS, 128], bf16, tag="gb")
                        nc.gpsimd.dma_gather(
                            gb[:, 0:ncols, :], view,
                            idx_t[:, cs * 8:(cs + ncols) * 8],
                            ncols * 128, ncols * 128, 128,
                            transpose=False, single_packet=False,
                            queue_num=call_i % max(nq, 1))
                        if skip_mm:
                            continue
                        for j0 in range(0, ncols, PTCOLS):
                            jn = min(PTCOLS, ncols - j0)
                            pt = ptg_p.tile([128, PTCOLS * 128], bf16,
                                            tag="ptg")
                            for j in range(jn):
                                nc.tensor.transpose(
                                    pt[0:do, j * 128:(j + 1) * 128],
                                    gb[:, j0 + j, 0:do],
                                    idnb_t[:, :])
                            # reduce runs of uniform (khat, s)
                            r0 = 0
                            while r0 < jn:
                                kd0, khat, s = cols[j0 + r0]
                                rn = 1
                                while (r0 + rn < jn
                                       and cols[j0 + r0 + rn][1] == khat
                                       and cols[j0 + r0 + rn][2] == s
                                       and cols[j0 + r0 + rn][0]
                                       == kd0 + rn * khat
                                       and (rn + 1) * khat <= 512):
                                    rn += 1
                                seg = pt[0:do, r0 * 128:(r0 + rn) * 128]
                                seg = seg.rearrange("p (n c) -> p n c", c=128)
                                seg = seg[:, :, 0:khat * s]
                                seg = seg.rearrange("p n (m q) -> p n m q",
                                                    q=s)
                                nd = rn * khat
                                if v == 0:
                                    zo = z[0:do, kd0:kd0 + nd].rearrange(
                                        "p (n m) -> p n m", m=khat)
                                    nc.vector.tensor_reduce(
                                        zo, seg,
                                        axis=mybir.AxisListType.X,
                                        op=mybir.AluOpType.add)
                                else:
                                    zo = zhi[0:do, kd0:kd0 + nd].rearrange(
                                        "p (n m) -> p n m", m=khat)
                                    with nc.allow_low_precision(
                                            reason="hi partial sums (~3 "
                                            "values) stored bf16"):
                                        nc.vector.tensor_reduce(
                                            zo, seg,
                                            axis=mybir.AxisListType.X,
                                            op=mybir.AluOpType.add)
                                r0 += rn
                    if dbg and k == 0 and rep == 0:
                        nc.sync.dma_start(z1_t[:, :], z[:, :])
                    # ---- y = relu(dinv*(z + g) + b)
                    y = y_p.tile([128, COLS], f32, tag="y")
                    if skip_gather or skip_mm:
                        nc.vector.tensor_copy(z[0:do, :], g_fm[0:do, :])
                    else:
                        nc.vector.tensor_add(z[0:do, :], z[0:do, :],
                                             zhi[0:do, :])
                        nc.vector.tensor_add(z[0:do, :], z[0:do, :],
                                             g_fm[0:do, :])
                    nc.vector.tensor_mul(z[0:do, :], z[0:do, :],
                                         dinv_t[0:do, :])
                    nc.scalar.activation(y[0:do, :], z[0:do, :],
                                         mybir.ActivationFunctionType.Relu,
                                         bias=B_t[k][0:do, :])
                    y_prev = y

                # ---- head: out = y4 @ Wl + bl
                osb = g_p.tile([C_OUT, COLS], f32, tag="g")
                for n in range(COLS // 512):
                    pm = pmm_p.tile([128, 512], f32, tag="pmm")
                    nc.tensor.matmul(pm[0:C_OUT, :], wl_t[0:H, 0:C_OUT],
                                     y_prev[0:H, n * 512:(n + 1) * 512])
                    nc.vector.tensor_scalar_add(osb[:, n * 512:(n + 1) * 512],
                                                pm[0:C_OUT, :], bl_t[:, 0:1])
                nc.sync.dma_start(out_t[:, :], osb[:, :])

    nc.compile()
    return nc


# ----------------------------------------------------------------------------
# host-side sharding glue
# ----------------------------------------------------------------------------
def _in_maps(prep, inputs):
    import ml_dtypes
    x = np.asarray(inputs["x"], np.float32)
    maps = []
    for c in range(M):
        nodes = prep["node_of"][c]
        x_fm = np.zeros((128, COLS), np.float32)
        x_fm[:, :NPC] = x[nodes[:NPC]].T
        dinv_b = np.broadcast_to(prep["dinv_cols"][c][None, :],
                                 (128, COLS)).copy()
        m = {
            "x_fm": x_fm,
            "idx_in": prep["idx_wrapped"][c],
            "dinv_in": dinv_b,
            "iden_in": np.eye(128, dtype=np.float32),
            "idnb_in": np.eye(128, dtype=np.float32).astype(ml_dtypes.bfloat16),
            "wl": np.asarray(inputs["Wl"], np.float32),
            "bl": np.asarray(inputs["bl"], np.float32).reshape(C_OUT, 1),
        }
        for k, (di, do) in enumerate(LAYER_DIMS):
            m[f"w{k+1}"] = np.asarray(inputs[f"W{k+1}"], np.float32)
            bcol = np.zeros((128, 1), np.float32)
            bcol[:do, 0] = np.asarray(inputs[f"b{k+1}"], np.float32)
            m[f"b{k+1}"] = bcol
        maps.append(m)
    return maps


def _unshard(prep, results):
    out = np.empty((N, C_OUT), np.float32)
    for c in range(M):
        o = results[c]["out_fm"]
        nodes = prep["node_of"][c]
        out[nodes[:NPC]] = o[:, :NPC].T
    return out


_CACHE = {}


def _get_program(edge_index, reps=1):
    key = (hash(edge_index.tobytes()), reps)
    if key not in _CACHE:
        prep = _prep(edge_index)
        nc = _build(prep, reps=reps)
        _CACHE[key] = (prep, nc)
    return _CACHE[key]


def kernel(**inputs):
    from concourse.bass_utils import run_bass_kernel_spmd

    edge_index = np.asarray(inputs["edge_index"], np.int32)
    reps = int(os.environ.get("GNN_REPS", "1"))
    prep, nc = _get_program(edge_index, reps)
    maps = _in_maps(prep, inputs)
    res = run_bass_kernel_spmd(nc, maps, core_ids=list(range(M)))
    kernel.last_results = res
    return _unshard(prep, res.results)


# revision 8
# speedup vs baseline: 1.2809x; 1.0537x over previous
"""GCN (4-layer message-passing + linear head) on 8 Trainium2 NeuronCores.

v4 design — non-transpose dma_gather + per-column PE transpose + DVE
segment reduce:

* Node partitioning as v2: nodes ranked by global in-degree, rank%8 = core,
  k_local = rank//8; per-node state feature-major in SBUF.
* Per layer the node-major bf16 table [40960, 128] is AllGathered as in v2.
* Messages are fetched with dma_gather(transpose=False): 256B contiguous
  rows land row i -> (partition i%128, free slot i//128) at DMA-bus speed
  (the v2 XBAR spray-transpose RX path ran ~7 ns/row).
* Edges are sorted by dst and packed into 128-row columns: per column,
  khat consecutive dsts x s slots (s = max edge count over the 8 cores for
  those dsts, khat*s <= 128); pad slots gather zero rows (spread over all
  zero rows to avoid a single-address HBM hotspot).
* Per column, one PE transpose (self-loading matmul against a bf16
  identity) lands the column feature-major in a PSUM tile (8 columns per
  bank, bf16); DVE tensor_reduce then sums each dst's contiguous s-slot
  stripe, writing z feature-major directly: lo-stream reduces write z,
  hi-stream reduces go to a scratch tile and are added.

Math (as v2): deg = indeg+1, dinv = deg^-1/2,
  y = relu(dinv*[segsum(g[src]) + g[v]] + b),  g = (dinv*x) @ W.
"""

import os
import sys
import numpy as np

for _p in ("/opt/trn_rl_repo",):
    if os.path.isdir(_p) and _p not in sys.path:
        sys.path.insert(0, _p)

# ----------------------------------------------------------------------------
# Problem constants
# ----------------------------------------------------------------------------
N = 40000
E = 640000
F_IN = 128
H = 64
C_OUT = 32
M = 8
NPC = 5000
TILES = 40
COLS = TILES * 128          # 5120
SH = COLS
TBL = SH * M                # 40960
VIEW_A = (0, 32768)
VIEW_B = (TBL - 32768, TBL)
CALL_COLS = 24              # max gather-call size in 128-row columns
PTCOLS = 8                  # transposed columns per PSUM tile (bf16)
LAYER_DIMS = [(F_IN, H), (H, 2 * H), (2 * H, 2 * H), (2 * H, H)]


def _row_in_shard(k):
    # row = k directly: high-degree (low-k) rows cluster at the shard front
    # for HBM row-buffer locality; zero rows [NPC, SH) are contiguous.
    return k


# ----------------------------------------------------------------------------
# CPU-side prep
# ----------------------------------------------------------------------------
def _prep(edge_index):
    src = np.asarray(edge_index[0], dtype=np.int64)
    dst = np.asarray(edge_index[1], dtype=np.int64)

    deg_in = np.bincount(dst, minlength=N)
    dinv = (1.0 / np.sqrt((deg_in + 1).astype(np.float32))).astype(np.float32)

    order = np.argsort(-deg_in, kind="stable")
    ranks = np.empty(N, np.int64)
    ranks[order] = np.arange(N)
    node_core = ranks % M
    k_local = ranks // M
    node_row = node_core * SH + _row_in_shard(k_local)

    r_src = node_row[src]
    is_hi = r_src >= VIEW_A[1]                    # stream: 0=lo, 1=hi
    kd = k_local[dst]
    core_e = node_core[dst]

    # per-dst slot need: max edge count over cores, per stream
    cnt = np.zeros((M, 2, COLS), np.int64)
    np.add.at(cnt, (core_e, is_hi.astype(np.int64), kd), 1)
    need = cnt.max(axis=0)                        # [2, COLS]

    # ---- column packing (continuous, per stream): khat consecutive dsts x
    # s slots, s = max need in the run, khat*s <= 128
    cols_meta = [[], []]    # per stream: (kd0, khat, s)
    dst_col = np.full((2, COLS), -1, np.int64)
    dst_stripe = np.zeros((2, COLS), np.int64)
    dst_s = np.zeros((2, COLS), np.int64)
    for v in range(2):
        p = 0
        while p < COLS:
            if need[v, p] == 0:
                p += 1
                continue
            s = int(need[v, p])
            khat = 1
            while p + khat < COLS and need[v, p + khat] > 0:
                s2 = max(s, int(need[v, p + khat]))
                if (khat + 1) * s2 <= 128:
                    khat += 1
                    s = s2
                else:
                    break
            ci = len(cols_meta[v])
            cols_meta[v].append((p, khat, s))
            for i in range(khat):
                dst_col[v, p + i] = ci
                dst_stripe[v, p + i] = i
                dst_s[v, p + i] = s
            p += khat

    # ---- call plan: lo and hi calls interleaved by dst progress (hi
    # reduces write a separate z_hi buffer, so ordering is free)
    specs = []              # (first_kd, v, local col range)
    for v in range(2):
        i = 0
        ncv = len(cols_meta[v])
        while i < ncv:
            nc_ = min(CALL_COLS, ncv - i)
            specs.append((cols_meta[v][i][0], v, i, nc_))
            i += nc_
    specs.sort()
    calls = []              # (view, col_start(global), ncols, [(kd0,khat,s)])
    ccur = 0
    col_gbase = [np.full(len(cols_meta[v]), -1, np.int64) for v in range(2)]
    for (_, v, i, nc_) in specs:
        for j in range(nc_):
            col_gbase[v][i + j] = ccur + j
        calls.append((v, ccur, nc_, cols_meta[v][i:i + nc_]))
        ccur += nc_
    NCOL = ccur
    IDXTOT = NCOL * 128

    # ---- per-edge slot assignment (vectorized)
    key = (((core_e * 2) + is_hi) * COLS + kd)
    order_e = np.lexsort((r_src, key))
    key_s = key[order_e]
    uniq, start_idx, counts = np.unique(key_s, return_index=True,
                                        return_counts=True)
    pos = np.arange(E) - np.repeat(start_idx, counts)

    kd_s = kd[order_e]
    v_s = is_hi[order_e].astype(np.int64)
    s_e = dst_s[v_s, kd_s]
    assert (pos < s_e).all(), "dst stream count exceeds slot count"
    cid = dst_col[v_s, kd_s]
    assert (cid >= 0).all()
    gc = np.empty(E, np.int64)
    for v in range(2):
        m = v_s == v
        gc[m] = col_gbase[v][cid[m]]
    assert (gc >= 0).all()
    flat_pos = gc * 128 + dst_stripe[v_s, kd_s] * s_e + pos

    view_base = np.where(v_s == 1, VIEW_B[0], VIEW_A[0])
    val = r_src[order_e] - view_base
    assert val.min() >= 0 and val.max() <= 32767

    # zero rows (nodes k>=NPC): spread pad reads over all of them
    zrows = []
    for c in range(M):
        for kk2 in range(NPC, SH):
            zrows.append(c * SH + _row_in_shard(kk2))
    zrows = np.array(sorted(zrows), np.int64)
    zr_a = zrows[(zrows >= VIEW_A[0]) & (zrows < VIEW_A[1])] - VIEW_A[0]
    zr_b = zrows[(zrows >= VIEW_B[0]) & (zrows < VIEW_B[1])] - VIEW_B[0]
    assert len(zr_a) > 100 and len(zr_b) > 100

    idx_flat = np.empty((M, IDXTOT), np.int16)
    pad_stream = np.empty(IDXTOT, np.int16)
    rr = np.arange(IDXTOT)
    for (v, cs, nc_, cols_gv) in calls:
        zr = zr_a if v == 0 else zr_b
        sl = slice(cs * 128, (cs + nc_) * 128)
        pad_stream[sl] = zr[rr[sl] % len(zr)].astype(np.int16)
    idx_flat[:] = pad_stream[None, :]
    idx_flat[core_e[order_e], flat_pos] = val.astype(np.int16)

    idx_wrapped = np.empty((M, 128, IDXTOT // 16), np.int16)
    for c in range(M):
        w = idx_flat[c].reshape(IDXTOT // 16, 16).T
        idx_wrapped[c] = np.tile(w, (8, 1))

    node_of = np.full((M, COLS), -1, np.int64)
    dinv_cols = np.zeros((M, COLS), np.float32)
    for c in range(M):
        nodes_c = order[c::M]
        node_of[c, :NPC] = nodes_c
        dinv_cols[c, :NPC] = dinv[nodes_c]

    return dict(
        node_of=node_of, dinv_cols=dinv_cols, idx_wrapped=idx_wrapped,
        idx_flat=idx_flat, calls=calls, cols_meta=cols_meta,
        NCOL=NCOL, IDXTOT=IDXTOT,
    )


# ----------------------------------------------------------------------------
# Bass/Tile program
# ----------------------------------------------------------------------------
def _build(prep, reps=1):
    import concourse.bass as bass
    import concourse.tile as tile
    from concourse import bacc, mybir

    calls = prep["calls"]
    IDXTOT = prep["IDXTOT"]
    f32 = mybir.dt.float32
    bf16 = mybir.dt.bfloat16

    single = bool(os.environ.get("GNN_SINGLE"))
    nq = int(os.environ.get("GNN_NQ", "4"))
    nc = bacc.Bacc("TRN2", target_bir_lowering=False, debug=False,
                   num_devices=1 if single else M,
                   num_swdge_queues=max(nq, 1))

    x_in = nc.dram_tensor("x_fm", [128, COLS], f32, kind="ExternalInput")
    idx_in = nc.dram_tensor("idx_in", [128, IDXTOT // 16], mybir.dt.int16,
                            kind="ExternalInput")
    dinv_in = nc.dram_tensor("dinv_in", [128, COLS], f32, kind="ExternalInput")
    iden_in = nc.dram_tensor("iden_in", [128, 128], f32, kind="ExternalInput")
    idnb_in = nc.dram_tensor("idnb_in", [128, 128], bf16, kind="ExternalInput")
    W_in, B_in = [], []
    for k, (di, do) in enumerate(LAYER_DIMS):
        W_in.append(nc.dram_tensor(f"w{k+1}", [di, do], f32, kind="ExternalInput"))
        B_in.append(nc.dram_tensor(f"b{k+1}", [128, 1], f32, kind="ExternalInput"))
    Wl_in = nc.dram_tensor("wl", [H, C_OUT], f32, kind="ExternalInput")
    bl_in = nc.dram_tensor("bl", [C_OUT, 1], f32, kind="ExternalInput")
    out_t = nc.dram_tensor("out_fm", [C_OUT, COLS], f32, kind="ExternalOutput")
    dbg = bool(os.environ.get("GNN_DEBUG_Z1"))
    z1_t = (nc.dram_tensor("z1_dbg", [128, COLS], f32, kind="ExternalOutput")
            if dbg else None)

    ag_in, table = [], []
    for k in range(4):
        ag_in.append(nc.dram_tensor(f"ag_in{k+1}", [SH, 128], bf16,
                                    kind="Internal"))
        table.append(nc.dram_tensor(f"table{k+1}", [TBL, 128], bf16,
                                    kind="Internal", addr_space="Shared"))

    with tile.TileContext(nc) as tc:
        import contextlib
        with contextlib.ExitStack() as ctx:
            const_p = ctx.enter_context(tc.tile_pool(name="const", bufs=1))
            y_p = ctx.enter_context(tc.tile_pool(name="y", bufs=2))
            scr_p = ctx.enter_context(tc.tile_pool(name="scr", bufs=2))
            g_p = ctx.enter_context(tc.tile_pool(name="g", bufs=1))
            gnm_p = ctx.enter_context(tc.tile_pool(name="gnm", bufs=1))
            gbuf_p = ctx.enter_context(tc.tile_pool(name="gbuf", bufs=7))
            zhi_p = ctx.enter_context(tc.tile_pool(name="zhi", bufs=1))
            pmm_p = ctx.enter_context(tc.tile_pool(name="pmm", bufs=2, space="PSUM"))
            ptr_p = ctx.enter_context(tc.tile_pool(name="ptr", bufs=3, space="PSUM"))
            ptg_p = ctx.enter_context(tc.tile_pool(name="ptg", bufs=3, space="PSUM"))

            idx_t = const_p.tile([128, IDXTOT // 16], mybir.dt.int16)
            nc.sync.dma_start(idx_t[:, :], idx_in[:, :])
            dinv_t = const_p.tile([128, COLS], f32)
            nc.sync.dma_start(dinv_t[:, :], dinv_in[:, :])
            iden_t = const_p.tile([128, 128], f32)
            nc.sync.dma_start(iden_t[:, :], iden_in[:, :])
            idnb_t = const_p.tile([128, 128], bf16)
            nc.sync.dma_start(idnb_t[:, :], idnb_in[:, :])
            W_t, B_t = [], []
            for k, (di, do) in enumerate(LAYER_DIMS):
                w = const_p.tile([di, do], f32, tag=f"w{k}")
                nc.sync.dma_start(w[:, :], W_in[k][:, :])
                W_t.append(w)
                b = const_p.tile([128, 1], f32, tag=f"b{k}")
                nc.sync.dma_start(b[:, :], B_in[k][:, :])
                B_t.append(b)
            wl_t = const_p.tile([H, C_OUT], f32, tag="wl")
            nc.sync.dma_start(wl_t[:, :], Wl_in[:, :])
            bl_t = const_p.tile([C_OUT, 1], f32, tag="bl")
            nc.sync.dma_start(bl_t[:, :], bl_in[:, :])
            x_t = y_p.tile([128, COLS], f32, tag="y")
            nc.sync.dma_start(x_t[:, :], x_in[:, :])

            skip_gather = os.environ.get("GNN_SKIP_GATHER")
            skip_mm = os.environ.get("GNN_SKIP_MM")

            for rep in range(reps):
                y_prev = None
                for k, (di, do) in enumerate(LAYER_DIMS):
                    # ---- x~ = dinv * y
                    xt = scr_p.tile([128, COLS], f32, tag="scr")
                    src_t = x_t if k == 0 else y_prev
                    nc.vector.tensor_mul(xt[0:di, :], src_t[0:di, :],
                                         dinv_t[0:di, :])
                    # ---- g = W^T @ x~  (feature-major [do, COLS])
                    g_fm = g_p.tile([128, COLS], f32, tag="g")
                    for n in range(COLS // 512):
                        pm = pmm_p.tile([128, 512], f32, tag="pmm")
                        nc.tensor.matmul(pm[0:do, :], W_t[k][0:di, 0:do],
                                         xt[0:di, n * 512:(n + 1) * 512])
                        nc.scalar.copy(g_fm[0:do, n * 512:(n + 1) * 512],
                                       pm[0:do, :])
                    # ---- node-major bf16 shard for the table
                    g_nm = gnm_p.tile([128, TILES, 128], bf16, tag="gnm")
                    if do < 128:
                        nc.vector.memset(g_nm[:, :, do:128], 0.0)
                    for t in range(TILES):
                        ps = ptr_p.tile([128, 128], f32, tag="ptr")
                        nc.tensor.transpose(ps[:, 0:do],
                                            g_fm[0:do, t * 128:(t + 1) * 128],
                                            iden_t[0:do, 0:do])
                        nc.scalar.copy(g_nm[:, t, 0:do], ps[:, 0:do])
                    # ---- publish + all-gather
                    nc.sync.dma_start(
                        ag_in[k][:, :].rearrange("(t p) f -> p t f", p=128),
                        g_nm[:, :, :])
                    if single:
                        nc.sync.dma_start(table[k][0:SH, :], ag_in[k][:, :])
                    elif not os.environ.get("GNN_SKIP_AG"):
                        nc.gpsimd.collective_compute(
                            "AllGather", mybir.AluOpType.bypass,
                            replica_groups=[list(range(M))],
                            ins=[ag_in[k][:, :]], outs=[table[k][:, :]],
                        )
                    # ---- gather + transpose + segment reduce
                    z = scr_p.tile([128, COLS], f32, tag="scr")
                    zhi = zhi_p.tile([128, COLS], bf16, tag="zhi")
                    if not skip_gather and not skip_mm:
                        nc.vector.memset(z[0:do, :], 0.0)
                        nc.vector.memset(zhi[0:do, :], 0.0)
                    for call_i, (v, cs, ncols, cols) in enumerate(calls):
                        if skip_gather:
                            continue
                        view = (table[k][VIEW_A[0]:VIEW_A[1], :] if v == 0
                                else table[k][VIEW_B[0]:VIEW_B[1], :])
                        gb = gbuf_p.tile([128, CALL_COLS, 128], bf16, tag="gb")
                        nc.gpsimd.dma_gather(
                            gb[:, 0:ncols, :], view,
                            idx_t[:, cs * 8:(cs + ncols) * 8],
                            ncols * 128, ncols * 128, 128,
                            transpose=False, single_packet=False,
                            queue_num=call_i % max(nq, 1))
                        if skip_mm:
                            continue
                        for j0 in range(0, ncols, PTCOLS):
                            jn = min(PTCOLS, ncols - j0)
                            pt = ptg_p.tile([128, PTCOLS * 128], bf16,
                                            tag="ptg")
                            for j in range(jn):
                                nc.tensor.transpose(
                                    pt[0:do, j * 128:(j + 1) * 128],
                                    gb[:, j0 + j, 0:do],
                                    idnb_t[:, :])
                            # reduce runs of uniform (khat, s)
                            r0 = 0
                            while r0 < jn:
                                kd0, khat, s = cols[j0 + r0]
                                rn = 1
                                while (r0 + rn < jn
                                       and cols[j0 + r0 + rn][1] == khat
                                       and cols[j0 + r0 + rn][2] == s
                                       and cols[j0 + r0 + rn][0]
                                       == kd0 + rn * khat
                                       and (rn + 1) * khat <= 512):
                                    rn += 1
                                seg = pt[0:do, r0 * 128:(r0 + rn) * 128]
                                seg = seg.rearrange("p (n c) -> p n c", c=128)
                                seg = seg[:, :, 0:khat * s]
                                seg = seg.rearrange("p n (m q) -> p n m q",
                                                    q=s)
                                nd = rn * khat
                                if v == 0:
                                    zo = z[0:do, kd0:kd0 + nd].rearrange(
                                        "p (n m) -> p n m", m=khat)
                                    nc.vector.tensor_reduce(
                                        zo, seg,
                                        axis=mybir.AxisListType.X,
                                        op=mybir.AluOpType.add)
                                else:
                                    zo = zhi[0:do, kd0:kd0 + nd].rearrange(
                                        "p (n m) -> p n m", m=khat)
                                    with nc.allow_low_precision(
                                            reason="hi partial sums (~3 "
                                            "values) stored bf16"):
                                        nc.vector.tensor_reduce(
                                            zo, seg,
                                            axis=mybir.AxisListType.X,
                                            op=mybir.AluOpType.add)
                                r0 += rn
                    if dbg and k == 0 and rep == 0:
                        nc.sync.dma_start(z1_t[:, :], z[:, :])
                    # ---- y = relu(dinv*(z + g) + b)
                    y = y_p.tile([128, COLS], f32, tag="y")
                    if skip_gather or skip_mm:
                        nc.vector.tensor_copy(z[0:do, :], g_fm[0:do, :])
                    else:
                        nc.vector.tensor_add(z[0:do, :], z[0:do, :],
                                             zhi[0:do, :])
                        nc.vector.tensor_add(z[0:do, :], z[0:do, :],
                                             g_fm[0:do, :])
                    nc.vector.tensor_mul(z[0:do, :], z[0:do, :],
                                         dinv_t[0:do, :])
                    nc.scalar.activation(y[0:do, :], z[0:do, :],
                                         mybir.ActivationFunctionType.Relu,
                                         bias=B_t[k][0:do, :])
                    y_prev = y

                # ---- head: out = y4 @ Wl + bl
                osb = g_p.tile([C_OUT, COLS], f32, tag="g")
                for n in range(COLS // 512):
                    pm = pmm_p.tile([128, 512], f32, tag="pmm")
                    nc.tensor.matmul(pm[0:C_OUT, :], wl_t[0:H, 0:C_OUT],
                                     y_prev[0:H, n * 512:(n + 1) * 512])
                    nc.vector.tensor_scalar_add(osb[:, n * 512:(n + 1) * 512],
                                                pm[0:C_OUT, :], bl_t[:, 0:1])
                nc.sync.dma_start(out_t[:, :], osb[:, :])

    nc.compile()
    return nc


# ----------------------------------------------------------------------------
# host-side sharding glue
# ----------------------------------------------------------------------------
def _in_maps(prep, inputs):
    import ml_dtypes
    x = np.asarray(inputs["x"], np.float32)
    maps = []
    for c in range(M):
        nodes = prep["node_of"][c]
        x_fm = np.zeros((128, COLS), np.float32)
        x_fm[:, :NPC] = x[nodes[:NPC]].T
        dinv_b = np.broadcast_to(prep["dinv_cols"][c][None, :],
                                 (128, COLS)).copy()
        m = {
            "x_fm": x_fm,
            "idx_in": prep["idx_wrapped"][c],
            "dinv_in": dinv_b,
            "iden_in": np.eye(128, dtype=np.float32),
            "idnb_in": np.eye(128, dtype=np.float32).astype(ml_dtypes.bfloat16),
            "wl": np.asarray(inputs["Wl"], np.float32),
            "bl": np.asarray(inputs["bl"], np.float32).reshape(C_OUT, 1),
        }
        for k, (di, do) in enumerate(LAYER_DIMS):
            m[f"w{k+1}"] = np.asarray(inputs[f"W{k+1}"], np.float32)
            bcol = np.zeros((128, 1), np.float32)
            bcol[:do, 0] = np.asarray(inputs[f"b{k+1}"], np.float32)
            m[f"b{k+1}"] = bcol
        maps.append(m)
    return maps


def _unshard(prep, results):
    out = np.empty((N, C_OUT), np.float32)
    for c in range(M):
        o = results[c]["out_fm"]
        nodes = prep["node_of"][c]
        out[nodes[:NPC]] = o[:, :NPC].T
    return out


_CACHE = {}


def _get_program(edge_index, reps=1):
    key = (hash(edge_index.tobytes()), reps)
    if key not in _CACHE:
        prep = _prep(edge_index)
        nc = _build(prep, reps=reps)
        _CACHE[key] = (prep, nc)
    return _CACHE[key]


def kernel(**inputs):
    from concourse.bass_utils import run_bass_kernel_spmd

    edge_index = np.asarray(inputs["edge_index"], np.int32)
    reps = int(os.environ.get("GNN_REPS", "1"))
    prep, nc = _get_program(edge_index, reps)
    maps = _in_maps(prep, inputs)
    res = run_bass_kernel_spmd(nc, maps, core_ids=list(range(M)))
    kernel.last_results = res
    return _unshard(prep, res.results)
